# revision 1
# baseline (speedup 1.0000x reference)
"""Arcee decoder layer on 8 TRN2 NeuronCores — tensor-parallel Bass kernel.

Sharding (8-way TP, transposed activation layout [hidden, seq] on device):
  - core c owns: q heads 4c..4c+3 + kv head c (GQA group), residual-stream
    rows 512c..512c+512, intermediate cols 2048c..2048c+2048.
  - RMSNorm trick: the un-normalized residual stream is AllGathered (bf16)
    with each core's partial sum-of-squares embedded as an extra row; every
    core derives the per-token rsqrt scale locally and the scale is folded
    into the next matmul's PSUM eviction (norm scale commutes with the
    matmul). ln weights are folded into the bf16 weight caches.
  - o_proj / down_proj emit transposed partials [4096,S]; bf16 ReduceScatter
    gives each core its hid-slice of the sum = its slice of the transposed
    outputs. Host reassembles by concat + transpose.
  - dtypes: all matmuls bf16 (f32 PSUM accumulation); residual stream and
    softmax statistics f32.
"""
import sys

sys.path.insert(0, "/opt/trn_rl_repo")

import math
import numpy as np

import concourse.bass as bass
import concourse.mybir as mybir
import concourse.tile as tile
from concourse import bacc
from concourse.bass_isa import ReduceOp
from concourse.masks import make_identity

F32 = mybir.dt.float32
BF16 = mybir.dt.bfloat16
I32 = mybir.dt.int32
AF = mybir.ActivationFunctionType
ALU = mybir.AluOpType

N_CORES = 8
S = 2048
HID = 4096
N_HEADS = 32
N_KV = 8
DHEAD = 128
INTER = 16384
EPS = 1e-5
THETA = 10000.0

HQ = N_HEADS // N_CORES          # 4 q heads per core
HID_SH = HID // N_CORES          # 512 residual rows per core
INT_SH = INTER // N_CORES        # 2048 intermediate per core
NJ = HQ + 2                      # qkv col tiles per core (4q + k + v)
QKV_COLS = NJ * DHEAD            # 768
P = 128
SC = 512                         # seq chunk (matmul moving dim)
NSC = S // SC                    # 4
NT_HID = HID // P                # 32
NT_HSH = HID_SH // P             # 4
NT_INT = INT_SH // P             # 16
BLK = HID_SH + 8                 # AG block rows: 512 payload + ssq row + pad
TWO_PI = 2.0 * math.pi


def build_graph():
    nc = bacc.Bacc(None, target_bir_lowering=False, debug=False)

    hT = nc.declare_dram_parameter("hT", [HID_SH, S], F32, isOutput=False)
    rT = nc.declare_dram_parameter("rT", [HID_SH, S], F32, isOutput=False)
    pos_in = nc.declare_dram_parameter("positions", [1, S], I32, isOutput=False)
    wqkv = nc.declare_dram_parameter("wqkv", [HID, QKV_COLS], F32, isOutput=False)
    wo = nc.declare_dram_parameter("wo", [HQ * DHEAD, HID], F32, isOutput=False)
    wup = nc.declare_dram_parameter("wup", [HID, INT_SH], F32, isOutput=False)
    wdn = nc.declare_dram_parameter("wdn", [INT_SH, HID], F32, isOutput=False)
    ln1 = nc.declare_dram_parameter("ln1", [P, NT_HID], F32, isOutput=False)
    ln2 = nc.declare_dram_parameter("ln2", [P, NT_HID], F32, isOutput=False)
    ln2s = nc.declare_dram_parameter("ln2s", [P, NT_HSH], F32, isOutput=False)
    out_res2 = nc.declare_dram_parameter("res2T", [HID_SH, S], F32, isOutput=True)
    out_mlp = nc.declare_dram_parameter("mlpT", [HID_SH, S], F32, isOutput=True)

    RG = [list(range(N_CORES))]
    inv_sqrt_d = 1.0 / math.sqrt(DHEAD)

    with tile.TileContext(nc) as tc:
        import contextlib
        with contextlib.ExitStack() as ctx:
            const = ctx.enter_context(tc.tile_pool(name="const", bufs=1))
            rowsb = ctx.enter_context(tc.tile_pool(name="rowsb", bufs=1))
            acc = ctx.enter_context(tc.tile_pool(name="acc", bufs=6, space="PSUM"))
            dram = ctx.enter_context(tc.tile_pool(name="dram", bufs=1, space="DRAM"))

            # ============ constants ============
            ident = const.tile([P, P], BF16)
            make_identity(nc, ident[:])
            ones_bf = const.tile([P, 1], BF16)
            nc.vector.memset(ones_bf[:], 1.0)
            ln1_sb = const.tile([P, NT_HID], F32)
            ln2s_sb = const.tile([P, NT_HSH], F32)
            nc.sync.dma_start(ln1_sb[:], ln1[:])
            nc.sync.dma_start(ln2s_sb[:], ln2s[:])
            cos2 = const.tile([P, S], BF16)
            sin_neg = const.tile([P, S], BF16)
            # causal masks for the 4 diagonal sk-tile offsets within a chunk:
            # mask[j]: keep (=1.0) where f - p - 128*j >= 0 else 0
            cmask = []
            for j in range(SC // P):
                mk = const.tile([P, SC], BF16, name=f"cmask{j}")
                nc.vector.memset(mk[:], 1.0)
                nc.gpsimd.affine_select(mk[:], mk[:], pattern=[[1, SC]],
                                        base=-j * P, channel_multiplier=-1,
                                        compare_op=ALU.is_ge, fill=0.0)
                cmask.append(mk)

            # DRAM scratch
            wo_c = dram.tile([P, HQ * HID], BF16, name="wo_c")
            SH = S // 2
            ag1_in = [dram.tile([BLK, SH], BF16, name=f"ag1_in{h}") for h in range(2)]
            ag1_out = [dram.tile([N_CORES * BLK, SH], BF16, name=f"ag1_out{h}",
                                 addr_space="Shared") for h in range(2)]
            ag2_in = [dram.tile([BLK, SH], BF16, name=f"ag2_in{h}") for h in range(2)]
            ag2_out = [dram.tile([N_CORES * BLK, SH], BF16, name=f"ag2_out{h}",
                                 addr_space="Shared") for h in range(2)]
            rs1_in = [dram.tile([HID, SC], BF16, name=f"rs1_in{sc}") for sc in range(NSC)]
            rs1_out = [dram.tile([HID_SH, SC], BF16, name=f"rs1_out{sc}")
                       for sc in range(NSC)]
            NQ = 4
            QROWS = HID // NQ                  # 1024 input rows per quarter
            QOUT = QROWS // N_CORES            # 128 output rows per quarter
            rs2_in = [dram.tile([QROWS, S], BF16, name=f"rs2_in{q}")
                      for q in range(NQ)]
            rs2_out = [dram.tile([QOUT, S], BF16, name=f"rs2_out{q}")
                       for q in range(NQ)]

            ag1_v = [t[:].rearrange("(c r) s -> c r s", r=BLK) for t in ag1_out]
            ag2_v = [t[:].rearrange("(c r) s -> c r s", r=BLK) for t in ag2_out]

            def half_cs(sc):
                return sc // 2, slice((sc % 2) * SC, (sc % 2 + 1) * SC)
            wo_v = wo_c[:].rearrange("p (a c) -> p a c", a=HQ)

            bcd = [dram.tile([1, SC], F32, name=f"bcd{i}") for i in range(8)]
            bcdb = [dram.tile([1, SC], BF16, name=f"bcdb{i}") for i in range(8)]
            _bci = [0]

            def row_broadcast(dst_ap, src_row):
                # dst[P, SC] <- broadcast of src_row[1, SC] via DRAM bounce;
                # bf16 dst goes through a bf16 bounce (DMA cannot cast)
                i = _bci[0] % len(bcd)
                _bci[0] += 1
                if dst_ap.dtype == BF16:
                    d = bcdb[i]
                else:
                    d = bcd[i]
                nc.sync.dma_start(d[:], src_row)
                nc.sync.dma_start(dst_ap, d[:].to_broadcast((P, SC)))

            def hg_src(ag_v, k, cs):
                # global hid tile k of the gathered (blocked) activation
                return ag_v[k // NT_HSH, (k % NT_HSH) * P:(k % NT_HSH + 1) * P, cs]

            # ============ rope tables (scoped scratch) ============
            with tc.tile_pool(name="tbl", bufs=1) as tbl:
                iot = tbl.tile([64, 1], I32)
                nc.gpsimd.iota(iot[:], pattern=[[1, 1]], base=0, channel_multiplier=1)
                iotf = tbl.tile([64, 1], F32)
                nc.vector.tensor_copy(iotf[:], iot[:])
                invf = tbl.tile([64, 1], F32)
                nc.scalar.activation(invf[:], iotf[:], AF.Exp,
                                     scale=-math.log(THETA) / 64.0)
                invf2pi = tbl.tile([64, 1], F32)
                nc.scalar.activation(invf2pi[:], invf[:], AF.Copy,
                                     scale=1.0 / TWO_PI)
                posi = tbl.tile([1, S], I32)
                nc.sync.dma_start(posi[:], pos_in[:])
                posf = tbl.tile([1, S], F32)
                nc.vector.tensor_copy(posf[:], posi[:])
                posb = tbl.tile([64, S], F32)
                nc.gpsimd.partition_broadcast(posb[:], posf[:])

                def range_reduce_sin(dst_bf, t_ap, negate=False):
                    # dst = sin(2*pi*t) via two-stage round-and-subtract
                    n1 = tbl.tile([64, S], I32, tag="rri", bufs=2, name="n1")
                    nc.vector.tensor_copy(n1[:], t_ap)
                    n1f = tbl.tile([64, S], F32, tag="rrf", bufs=2, name="n1f")
                    nc.vector.tensor_copy(n1f[:], n1[:])
                    f1 = tbl.tile([64, S], F32, tag="rrg", bufs=2, name="f1")
                    nc.vector.tensor_tensor(f1[:], t_ap, n1f[:], ALU.subtract)
                    n2 = tbl.tile([64, S], I32, tag="rri", bufs=2, name="n2")
                    nc.vector.tensor_copy(n2[:], f1[:])
                    n2f = tbl.tile([64, S], F32, tag="rrf", bufs=2, name="n2f")
                    nc.vector.tensor_copy(n2f[:], n2[:])
                    f2 = tbl.tile([64, S], F32, tag="rrg", bufs=2, name="f2")
                    nc.vector.tensor_tensor(f2[:], f1[:], n2f[:], ALU.subtract)
                    nc.scalar.activation(dst_bf, f2[:], AF.Sin,
                                         scale=-TWO_PI if negate else TWO_PI)

                tfrac = tbl.tile([64, S], F32)
                nc.scalar.activation(tfrac[:], posb[:], AF.Copy, scale=invf2pi[:])
                sinb = tbl.tile([64, S], BF16)
                sinnb = tbl.tile([64, S], BF16)
                range_reduce_sin(sinb[:], tfrac[:])
                range_reduce_sin(sinnb[:], tfrac[:], negate=True)
                tfrac2 = tbl.tile([64, S], F32)
                nc.scalar.activation(tfrac2[:], tfrac[:], AF.Copy, bias=0.25)
                cosb = tbl.tile([64, S], BF16)
                range_reduce_sin(cosb[:], tfrac2[:])
                nc.sync.dma_start(cos2[:64, :], cosb[:])
                nc.sync.dma_start(cos2[64:, :], cosb[:])
                nc.sync.dma_start(sin_neg[:64, :], sinnb[:])
                nc.sync.dma_start(sin_neg[64:, :], sinb[:])

            # ================== attention era ==================
            with tc.tile_pool(name="apersist", bufs=1) as apersist, \
                 tc.tile_pool(name="awork", bufs=1) as awork, \
                 tc.tile_pool(name="wstr", bufs=1) as wstr, \
                 tc.tile_pool(name="rowps", bufs=1, space="PSUM") as rowps, \
                 tc.tile_pool(name="tpps", bufs=1, space="PSUM") as tpps:

                _cnt = [0]

                def t2k(tag="t2k", bufs=7):
                    _cnt[0] += 1
                    return awork.tile([P, SC], F32, tag=tag, bufs=bufs,
                                      name=f"t_{_cnt[0]}")

                def t1k(tag="t1k", bufs=7):
                    _cnt[0] += 1
                    return awork.tile([P, SC], BF16, tag=tag, bufs=bufs,
                                      name=f"t_{_cnt[0]}")

                # ---- phase 1: x = h + r (chunked), ssq row, ag1_in (bf16)
                ssq1 = awork.tile([1, S], F32, name="ssq1")
                for sc in range(NSC):
                    cs = slice(sc * SC, (sc + 1) * SC)
                    ps = rowps.tile([1, SC], F32, tag="row", name=f"ssq1p{sc}")
                    for i in range(NT_HSH):
                        a = t2k()
                        b = t2k()
                        nc.sync.dma_start(a[:], hT[i * P:(i + 1) * P, cs])
                        nc.sync.dma_start(b[:], rT[i * P:(i + 1) * P, cs])
                        xt = t2k()
                        nc.vector.tensor_tensor(xt[:], a[:], b[:], ALU.add)
                        xb = t1k()
                        nc.vector.tensor_copy(xb[:], xt[:])
                        hh, hcs = half_cs(sc)
                        nc.sync.dma_start(ag1_in[hh][i * P:(i + 1) * P, hcs], xb[:])
                        sq = t1k(tag="sq", bufs=2)
                        nc.scalar.activation(sq[:], xt[:], AF.Square)
                        nc.tensor.matmul(ps[:], ones_bf[:], sq[:],
                                         start=(i == 0), stop=(i == NT_HSH - 1))
                    nc.vector.tensor_copy(ssq1[:, cs], ps[:])
                    if sc % 2 == 1:
                        hh = sc // 2
                        ssq1b = awork.tile([1, SH], BF16, tag="ssq1b", bufs=2,
                                           name=f"ssq1b{hh}")
                        nc.vector.tensor_copy(ssq1b[:],
                                              ssq1[:, hh * SH:(hh + 1) * SH])
                        nc.sync.dma_start(ag1_in[hh][HID_SH:HID_SH + 1, :], ssq1b[:])
                        nc.gpsimd.collective_compute(
                            "AllGather", ALU.bypass, replica_groups=RG,
                            ins=[ag1_in[hh][:].opt()], outs=[ag1_out[hh][:].opt()])

                # ---- weight caches (emitted after AG1 so phase-1 DMAs win)
                def build_cache(src, n_row_tiles, n_cols, dst, ln_col, eng):
                    CB = min(n_cols, 768)
                    for k in range(n_row_tiles):
                        for c0 in range(0, n_cols, CB):
                            wf = wstr.tile([P, CB], F32, tag="cbf", bufs=2,
                                           name=f"cb_{dst.tensor.name}_{k}_{c0}")
                            nc.scalar.dma_start(wf[:, :min(CB, n_cols - c0)],
                                                src[k * P:(k + 1) * P,
                                                    c0:c0 + min(CB, n_cols - c0)])
                            wb = wstr.tile([P, CB], BF16, tag="cbb", bufs=2,
                                           name=f"cc_{dst.tensor.name}_{k}_{c0}")
                            w = min(CB, n_cols - c0)
                            if ln_col is not None:
                                nc.scalar.activation(wb[:, :w], wf[:, :w], AF.Copy,
                                                     scale=ln_col[:, k:k + 1])
                            else:
                                eng.tensor_copy(wb[:, :w], wf[:, :w])
                            nc.scalar.dma_start(
                                dst[:, k * n_cols + c0:k * n_cols + c0 + w],
                                wb[:, :w])

                # wqkv: convert once straight into persistent SBUF (ln1 folded)
                wqkv_sb = [apersist.tile([P, NT_HID, P], BF16, name=f"wqsb{j}")
                           for j in range(NJ)]
                for k in range(NT_HID):
                    wfq = wstr.tile([P, QKV_COLS], F32, tag="cbf", bufs=2,
                                    name=f"wfq{k}")
                    nc.scalar.dma_start(wfq[:], wqkv[k * P:(k + 1) * P, :])
                    for j in range(NJ):
                        nc.scalar.activation(wqkv_sb[j][:, k, :],
                                             wfq[:, j * P:(j + 1) * P], AF.Copy,
                                             scale=ln1_sb[:, k:k + 1])
                build_cache(wo, HQ, HID, wo_c, None, nc.vector)

                # per-chunk norm1 scale rows (from gathered ssq partials)
                s1b = apersist.tile([P, S], BF16, name="s1b")  # rsqrt scale bcast
                for sc in range(NSC):
                    cs = slice(sc * SC, (sc + 1) * SC)
                    hh, hcs = half_cs(sc)
                    srows_b = awork.tile([8, SC], BF16, tag="srb", bufs=1,
                                         name=f"sr1b{sc}")
                    nc.gpsimd.dma_start(srows_b[:], ag1_v[hh][:, HID_SH, hcs])
                    srows = awork.tile([8, SC], F32, tag="srf", bufs=1,
                                       name=f"sr1f{sc}")
                    nc.vector.tensor_copy(srows[:], srows_b[:])
                    ssum = awork.tile([8, SC], F32, tag="ssum", bufs=1,
                                      name=f"ss1{sc}")
                    nc.gpsimd.partition_all_reduce(ssum[:], srows[:], channels=8,
                                                   reduce_op=ReduceOp.add)
                    var = awork.tile([1, SC], F32, tag="var", bufs=2, name=f"v1{sc}")
                    nc.scalar.activation(var[:], ssum[:1, :], AF.Copy,
                                         scale=1.0 / HID, bias=EPS)
                    nc.vector.reciprocal(var[:], var[:])
                    varb = awork.tile([1, SC], BF16, tag="varb", bufs=2,
                                      name=f"v1b{sc}")
                    nc.scalar.activation(varb[:], var[:], AF.Sqrt)
                    row_broadcast(s1b[:, cs], varb[:])

                # k and v keep full-S persistent tiles; q tiles rotate per chunk
                kT = apersist.tile([P, S], BF16, name="kT")
                vT = apersist.tile([P, S], BF16, name="vT")

                qcs = {}

                def emit_qkv(sc):
                    cs = slice(sc * SC, (sc + 1) * SC)
                    cs = slice(sc * SC, (sc + 1) * SC)
                    # ---- qkv chunk (scale1 folded into eviction)
                    hgb = []
                    hh, hcs = half_cs(sc)
                    for cb in range(N_CORES):
                        g = awork.tile([P, NT_HSH, SC], BF16, tag="hg", bufs=8,
                                       name=f"hg{cb}_{sc}")
                        nc.gpsimd.dma_start(
                            g[:], ag1_v[hh][cb, :HID_SH, hcs].rearrange(
                                "(t p) s -> p t s", p=P))
                        hgb.append(g)
                    hg = [hgb[k // NT_HSH][:, k % NT_HSH, :] for k in range(NT_HID)]
                    qc = {}
                    for j in range(NJ):
                        ps = acc.tile([P, SC], F32, tag="acc", name=f"qk{j}_{sc}")
                        for k in range(NT_HID):
                            nc.tensor.matmul(ps[:], wqkv_sb[j][:, k, :], hg[k],
                                             start=(k == 0), stop=(k == NT_HID - 1))
                        if j < HQ:
                            dst = awork.tile([P, SC], BF16, tag="qc", bufs=10,
                                             name=f"qc{j}_{sc}")
                            qc[j] = dst
                            nc.vector.tensor_tensor(dst[:], ps[:], s1b[:, cs],
                                                    ALU.mult)
                        else:
                            dst = kT if j == HQ else vT
                            nc.vector.tensor_tensor(dst[:, cs], ps[:], s1b[:, cs],
                                                    ALU.mult)

                    qcs[sc] = qc

                def emit_attn(sc):
                    cs = slice(sc * SC, (sc + 1) * SC)
                    qc = qcs[sc]
                    # ---- rope on q tiles and k tile (bf16, chunk cols)
                    for j in range(HQ + 1):
                        tv = qc[j][:] if j < HQ else kT[:, cs]
                        swp = t1k()
                        nc.sync.dma_start(swp[:64, :], tv[64:, :])
                        nc.sync.dma_start(swp[64:, :], tv[:64, :])
                        m1 = t1k()
                        nc.vector.tensor_tensor(m1[:], tv, cos2[:, cs], ALU.mult)
                        m2 = t1k()
                        nc.vector.tensor_tensor(m2[:], swp[:], sin_neg[:, cs], ALU.mult)
                        nc.vector.tensor_tensor(tv, m1[:], m2[:], ALU.add)

                    # ---- v transpose in place (block-transposed v)
                    for t in range(sc * (SC // P), (sc + 1) * (SC // P)):
                        pst = tpps.tile([P, P], BF16, tag="tp", name=f"tp{t}")
                        nc.tensor.transpose(pst[:], vT[:, t * P:(t + 1) * P],
                                            ident[:])
                        nc.vector.tensor_copy(vT[:, t * P:(t + 1) * P], pst[:])

                    # ---- attention (4 heads x this chunk)
                    nsk = (sc + 1) * (SC // P)
                    attnT = {}
                    for h in range(HQ):
                        pv = acc.tile([P, SC], F32, tag="acc", name=f"pv{h}_{sc}")
                        rs = rowps.tile([1, SC], F32, tag="row", name=f"rs{h}_{sc}")
                        for skt in range(nsk):
                            sps = acc.tile([P, SC], F32, tag="acc",
                                           name=f"s{h}_{sc}_{skt}")
                            nc.tensor.matmul(sps[:],
                                             kT[:, skt * P:(skt + 1) * P],
                                             qc[h][:], start=True, stop=True)
                            ex = t1k(tag="ex", bufs=5)
                            nc.scalar.activation(ex[:], sps[:], AF.Exp,
                                                 scale=inv_sqrt_d)
                            if skt >= 4 * sc:
                                nc.vector.tensor_tensor(ex[:], ex[:],
                                                        cmask[skt - 4 * sc][:],
                                                        ALU.mult)
                            nc.tensor.matmul(rs[:], ones_bf[:], ex[:],
                                             start=(skt == 0), stop=(skt == nsk - 1))
                            nc.tensor.matmul(pv[:],
                                             vT[:, skt * P:(skt + 1) * P],
                                             ex[:], start=(skt == 0),
                                             stop=(skt == nsk - 1))
                        rcp = awork.tile([1, SC], F32, tag="rcp", bufs=2,
                                         name=f"rcp{h}_{sc}")
                        nc.vector.reciprocal(rcp[:], rs[:])
                        rcpb = t2k(tag="rcpb", bufs=2)
                        row_broadcast(rcpb[:], rcp[:])
                        at = awork.tile([P, SC], BF16, tag="attnT", bufs=6,
                                        name=f"at{h}_{sc}")
                        nc.vector.tensor_tensor(at[:], pv[:], rcpb[:], ALU.mult)
                        attnT[h] = at

                    # ---- o_proj chunk -> bf16 ReduceScatter
                    for m in range(NT_HID):
                        wm = wstr.tile([P, HQ, P], BF16, tag="wos", bufs=2,
                                       name=f"wm{m}_{sc}")
                        nc.scalar.dma_start(wm[:], wo_v[:, :, m * P:(m + 1) * P])
                        ps = acc.tile([P, SC], F32, tag="acc", name=f"o{m}_{sc}")
                        for a in range(HQ):
                            nc.tensor.matmul(ps[:], wm[:, a, :], attnT[a][:],
                                             start=(a == 0), stop=(a == HQ - 1))
                        ev = t1k(tag="oev", bufs=2)
                        nc.vector.tensor_copy(ev[:], ps[:])
                        nc.sync.dma_start(rs1_in[sc][m * P:(m + 1) * P, :], ev[:])
                    nc.gpsimd.collective_compute(
                        "ReduceScatter", ALU.add, replica_groups=RG,
                        ins=[rs1_in[sc][:].opt()], outs=[rs1_out[sc][:].opt()])

                    # ---- residual2 chunk -> raw bf16 + ssq row into ag2_in
                    ps2 = rowps.tile([1, SC], F32, tag="row", name=f"ssq2_{sc}")
                    for i in range(NT_HSH):
                        o = t1k(tag="r2ld", bufs=2)
                        nc.gpsimd.dma_start(o[:], rs1_out[sc][i * P:(i + 1) * P, :])
                        hh, hcs = half_cs(sc)
                        xr = t1k(tag="xr", bufs=3)
                        nc.sync.dma_start(xr[:], ag1_in[hh][i * P:(i + 1) * P, hcs])
                        r2t = t2k(tag="r2", bufs=4)
                        nc.vector.tensor_tensor(r2t[:], o[:], xr[:], ALU.add)
                        nc.sync.dma_start(out_res2[i * P:(i + 1) * P, cs], r2t[:])
                        r2b = t1k()
                        nc.scalar.activation(r2b[:], r2t[:], AF.Copy,
                                             scale=ln2s_sb[:, i:i + 1])
                        hh, hcs = half_cs(sc)
                        nc.sync.dma_start(ag2_in[hh][i * P:(i + 1) * P, hcs], r2b[:])
                        sq = t1k(tag="sq", bufs=2)
                        nc.scalar.activation(sq[:], r2t[:], AF.Square)
                        nc.tensor.matmul(ps2[:], ones_bf[:], sq[:],
                                         start=(i == 0), stop=(i == NT_HSH - 1))
                    ssq2 = awork.tile([1, SC], BF16, tag="ssq2", bufs=2,
                                      name=f"sq2_{sc}")
                    nc.vector.tensor_copy(ssq2[:], ps2[:])
                    hh, hcs = half_cs(sc)
                    nc.sync.dma_start(ag2_in[hh][HID_SH:HID_SH + 1, hcs], ssq2[:])
                    if sc % 2 == 1:
                        nc.gpsimd.collective_compute(
                            "AllGather", ALU.bypass, replica_groups=RG,
                            ins=[ag2_in[hh][:].opt()], outs=[ag2_out[hh][:].opt()])


                emit_qkv(0)
                for sc in range(NSC):
                    if sc + 1 < NSC:
                        emit_qkv(sc + 1)
                    emit_attn(sc)

            # ================== MLP era ==================
            with tc.tile_pool(name="mpersist", bufs=1) as mpersist, \
                 tc.tile_pool(name="mwork", bufs=1) as mwork, \
                 tc.tile_pool(name="mstr", bufs=1) as mstr, \
                 tc.tile_pool(name="macc", bufs=2, space="PSUM") as macc:
                # per-chunk 1/var rows (scale2^2 fold for down eviction)
                s2b = mpersist.tile([P, S], F32, name="s2b")
                for sc in range(NSC):
                    cs = slice(sc * SC, (sc + 1) * SC)
                    hh, hcs = half_cs(sc)
                    srows_b = mwork.tile([8, SC], BF16, tag="srb", bufs=1,
                                         name=f"sr2b{sc}")
                    nc.gpsimd.dma_start(srows_b[:], ag2_v[hh][:, HID_SH, hcs])
                    srows = mwork.tile([8, SC], F32, tag="srf", bufs=1,
                                       name=f"sr2f{sc}")
                    nc.vector.tensor_copy(srows[:], srows_b[:])
                    ssum = mwork.tile([8, SC], F32, tag="ssum", bufs=1,
                                      name=f"ss2{sc}")
                    nc.gpsimd.partition_all_reduce(ssum[:], srows[:], channels=8,
                                                   reduce_op=ReduceOp.add)
                    var = mwork.tile([1, SC], F32, tag="var", bufs=2, name=f"v2{sc}")
                    nc.scalar.activation(var[:], ssum[:1, :], AF.Copy,
                                         scale=1.0 / HID, bias=EPS)
                    nc.vector.reciprocal(var[:], var[:])  # = scale2^2
                    row_broadcast(s2b[:, cs], var[:])

                uT = [mpersist.tile([P, S], BF16, name=f"uT{it}")
                      for it in range(NT_INT)]
                for half in range(2):
                    hs = [half * 2, half * 2 + 1]
                    h2gb = []
                    for cb in range(N_CORES):
                        g = mwork.tile([P, NT_HSH, 2 * SC], BF16, tag=f"h2g{cb}",
                                       bufs=1, name=f"h2g{cb}_{half}")
                        nc.sync.dma_start(
                            g[:], ag2_v[half][cb, :HID_SH, :].rearrange(
                                "(t p) s -> p t s", p=P))
                        h2gb.append(g)
                    h2g = [h2gb[k // NT_HSH][:, k % NT_HSH, :]
                           for k in range(NT_HID)]
                    wup_vv = wup[:].rearrange("(k p) c -> p k c", p=P)
                    for it in range(NT_INT):
                        KQ = NT_HID // 4
                        wts = []
                        for khh in range(4):
                            wfu = mstr.tile([P, KQ, P], F32, tag="wupf", bufs=2,
                                            name=f"wf{it}_{half}_{khh}")
                            nc.scalar.dma_start(
                                wfu[:], wup_vv[:, khh * KQ:(khh + 1) * KQ,
                                               it * P:(it + 1) * P])
                            wtb = mstr.tile([P, KQ, P], BF16, tag="wups", bufs=5,
                                            name=f"wt{it}_{half}_{khh}")
                            nc.scalar.activation(wtb[:], wfu[:], AF.Copy)
                            wts.append(wtb)
                        for ci, sc_ in enumerate(hs):
                            ps = acc.tile([P, SC], F32, tag="acc",
                                          name=f"up{it}_{sc_}")
                            for k in range(NT_HID):
                                wk = wts[k // KQ][:, k % KQ, :]
                                nc.tensor.matmul(ps[:], wk,
                                                 h2g[k][:, ci * SC:(ci + 1) * SC],
                                                 start=(k == 0),
                                                 stop=(k == NT_HID - 1)) \
                                    if False else \
                                    nc.tensor.matmul(ps[:], wk,
                                                     h2gb[k // NT_HSH][:, k % NT_HSH,
                                                                       ci * SC:(ci + 1) * SC],
                                                     start=(k == 0),
                                                     stop=(k == NT_HID - 1))
                            rl = mwork.tile([P, SC], F32, tag="relu", bufs=2,
                                            name=f"rl{it}_{sc_}")
                            nc.scalar.activation(rl[:], ps[:], AF.Relu)
                            nc.vector.tensor_tensor(
                                uT[it][:, sc_ * SC:(sc_ + 1) * SC], rl[:], rl[:],
                                ALU.mult)

                wdn_v = wdn[:].rearrange("(t p) c -> p t c", p=P)
                MQ = NT_HID // 4  # m tiles per RS2 quarter
                for m in range(NT_HID):
                    wdn_t = []
                    IH = NT_INT // 2
                    for ih in range(2):
                        wf = mstr.tile([P, IH, P], F32, tag="wdnf", bufs=2,
                                       name=f"wf{m}_{ih}")
                        nc.scalar.dma_start(
                            wf[:], wdn_v[:, ih * IH:(ih + 1) * IH,
                                         m * P:(m + 1) * P])
                        wb = mstr.tile([P, IH, P], BF16, tag="wdnb", bufs=2,
                                       name=f"wb{m}_{ih}")
                        nc.scalar.activation(wb[:], wf[:], AF.Copy)
                        wdn_t.append(wb)
                    for sc in range(NSC):
                        cs = slice(sc * SC, (sc + 1) * SC)
                        ps = macc.tile([P, SC], F32, tag="macc", name=f"dn{m}_{sc}")
                        for it in range(NT_INT):
                            nc.tensor.matmul(ps[:], wdn_t[it // IH][:, it % IH, :],
                                             uT[it][:, cs],
                                             start=(it == 0), stop=(it == NT_INT - 1))
                        ev = mwork.tile([P, SC], BF16, tag="dnev", bufs=3,
                                        name=f"dev{m}_{sc}")
                        nc.vector.tensor_tensor(ev[:], ps[:], s2b[:, cs], ALU.mult)
                        q_, mq_ = divmod(m, MQ)
                        nc.sync.dma_start(rs2_in[q_][mq_ * P:(mq_ + 1) * P, cs],
                                          ev[:])
                    if (m + 1) % MQ == 0:
                        q = m // MQ
                        nc.gpsimd.collective_compute(
                            "ReduceScatter", ALU.add, replica_groups=RG,
                            ins=[rs2_in[q][:].opt()],
                            outs=[rs2_out[q][:].opt()])
                        # out_mlp rows [128q:128q+128] hold this core's quarter-q
                        # slice (global hid rows 1024q + 128*core); host remaps.
                        nc.gpsimd.dma_start(out_mlp[q * P:(q + 1) * P, :],
                                            rs2_out[q][:])

    nc.compile()
    return nc


def shard_inputs(positions, hidden_states, residual, qkv_w, o_w, up_w, down_w,
                 ln1_w, ln2_w):
    hTf = np.ascontiguousarray(np.asarray(hidden_states).reshape(S, HID).T)
    rTf = np.ascontiguousarray(np.asarray(residual).reshape(S, HID).T)
    pos = np.ascontiguousarray(np.asarray(positions).reshape(1, S))
    ln1_t = np.ascontiguousarray(np.asarray(ln1_w).reshape(NT_HID, P).T)  # [128,32]
    ln2_t = np.ascontiguousarray(np.asarray(ln2_w).reshape(NT_HID, P).T)
    q_size = N_HEADS * DHEAD
    kv = N_KV * DHEAD
    in_maps = []
    for c in range(N_CORES):
        wqkv_c = np.concatenate([
            qkv_w[:, c * HQ * DHEAD:(c + 1) * HQ * DHEAD],
            qkv_w[:, q_size + c * DHEAD:q_size + (c + 1) * DHEAD],
            qkv_w[:, q_size + kv + c * DHEAD:q_size + kv + (c + 1) * DHEAD],
        ], axis=1)
        in_maps.append({
            "hT": np.ascontiguousarray(hTf[c * HID_SH:(c + 1) * HID_SH]),
            "rT": np.ascontiguousarray(rTf[c * HID_SH:(c + 1) * HID_SH]),
            "positions": pos,
            "wqkv": np.ascontiguousarray(wqkv_c),
            "wo": np.ascontiguousarray(o_w[c * HQ * DHEAD:(c + 1) * HQ * DHEAD, :]),
            "wup": np.ascontiguousarray(up_w[:, c * INT_SH:(c + 1) * INT_SH]),
            "wdn": np.ascontiguousarray(down_w[c * INT_SH:(c + 1) * INT_SH, :]),
            "ln1": ln1_t,
            "ln2": ln2_t,
            "ln2s": np.ascontiguousarray(ln2_t[:, c * NT_HSH:(c + 1) * NT_HSH]),
        })
    return in_maps


_CACHE = {}


def kernel(**inputs):
    from concourse.bass_utils import run_bass_kernel_spmd
    if "nc" not in _CACHE:
        _CACHE["nc"] = build_graph()
    nc = _CACHE["nc"]
    in_maps = shard_inputs(**{k: np.asarray(v) for k, v in inputs.items()})
    res = run_bass_kernel_spmd(nc, in_maps, core_ids=list(range(N_CORES)),
                               trace=False)
    res2T = np.concatenate([res.results[c]["res2T"] for c in range(N_CORES)], axis=0)
    mlpT = np.empty((HID, S), np.float32)
    for c in range(N_CORES):
        mt = res.results[c]["mlpT"]
        for q in range(4):
            mlpT[q * 1024 + c * 128:q * 1024 + (c + 1) * 128] = \
                mt[q * 128:(q + 1) * 128]
    mlp_out = np.ascontiguousarray(mlpT.T).reshape(1, S, HID)
    residual2 = np.ascontiguousarray(res2T.T).reshape(1, S, HID)
    return mlp_out, residual2



# revision 13
# speedup vs baseline: 1.1244x; 1.1244x over previous
"""Arcee decoder layer on 8 TRN2 NeuronCores — tensor-parallel Bass kernel v2.

Sharding (8-way TP, transposed activation layout [hidden, seq] on device):
  - core c owns: q heads 4c..4c+3 + kv head c (GQA group), residual-stream
    rows 512c..512c+512, intermediate cols 2048c..2048c+2048.
  - Weights are cast to bf16 and packed into SBUF-tile layout on the host
    (ln1 folded into qkv weights, ln2 folded into up weights).
  - RMSNorm trick: the un-normalized residual stream is AllGathered (bf16,
    per-chunk) with each core's partial sum-of-squares embedded as an extra
    row; every core derives the per-token rsqrt scale locally; the scale is
    applied at qkv eviction (attn) / folded into the down eviction (mlp).
  - o_proj emits transposed partials [4096,S-chunk] -> bf16 ReduceScatter
    per chunk; down_proj -> 8 ReduceScatters of [1024, 1024].
  - Emission order is software-pipelined: qkv for all 4 seq chunks first
    (PE-dense), then attention chunk pairs woven with o_proj / residual2 /
    up-proj streams so the PE never head-of-line blocks on collectives.
"""
import sys

sys.path.insert(0, "/opt/trn_rl_repo")

import math
import numpy as np

import concourse.bass as bass
import concourse.mybir as mybir
import concourse.tile as tile
from concourse import bacc
from concourse.bass_isa import ReduceOp
from concourse.masks import make_identity

F32 = mybir.dt.float32
BF16 = mybir.dt.bfloat16
I32 = mybir.dt.int32
AF = mybir.ActivationFunctionType
ALU = mybir.AluOpType

N_CORES = 8
S = 2048
HID = 4096
N_HEADS = 32
N_KV = 8
DHEAD = 128
INTER = 16384
EPS = 1e-5
THETA = 10000.0

HQ = N_HEADS // N_CORES          # 4 q heads per core
HID_SH = HID // N_CORES          # 512 residual rows per core
INT_SH = INTER // N_CORES        # 2048 intermediate per core
NJ = HQ + 2                      # qkv col tiles per core (4q + k + v)
P = 128
SC = 512                         # seq chunk
NSC = S // SC                    # 4
SH = S // 2                      # token half
NT_HID = HID // P                # 32
NT_HSH = HID_SH // P             # 4
NT_INT = INT_SH // P             # 16
BLK = HID_SH + 8                 # AG block rows: 512 payload + ssq row + pad
TWO_PI = 2.0 * math.pi

DIAG_TRIM = True                 # restrict diagonal score tiles to valid cols


def weave(*streams):
    """Round-robin generators: (gen, weight) or gen (weight 1)."""
    live = []
    for s in streams:
        if isinstance(s, tuple):
            live.append([s[0], s[1]])
        else:
            live.append([s, 1])
    while live:
        for ent in list(live):
            g, w = ent
            for _ in range(w):
                try:
                    next(g)
                except StopIteration:
                    live.remove(ent)
                    break


def build_graph():
    nc = bacc.Bacc(None, target_bir_lowering=False, debug=False)

    hT = nc.declare_dram_parameter("hT", [HID_SH, S], F32, isOutput=False)
    rT = nc.declare_dram_parameter("rT", [HID_SH, S], F32, isOutput=False)
    pos_in = nc.declare_dram_parameter("positions", [1, S], I32, isOutput=False)
    wqkv = nc.declare_dram_parameter("wqkv", [NJ, P, NT_HID * P], BF16,
                                     isOutput=False)
    wo = nc.declare_dram_parameter("wo", [P, HQ * NT_HID * P], BF16,
                                   isOutput=False)
    wup = nc.declare_dram_parameter("wup", [NT_INT, P, NT_HID * P], BF16,
                                    isOutput=False)
    wdn = nc.declare_dram_parameter("wdn", [NT_HID, P, NT_INT * P], BF16,
                                    isOutput=False)
    out_res2 = nc.declare_dram_parameter("res2T", [HID_SH, S], F32,
                                         isOutput=True)
    out_mlp = nc.declare_dram_parameter("mlpT", [HID_SH, S], F32,
                                        isOutput=True)

    RG = [list(range(N_CORES))]
    inv_sqrt_d = 1.0 / math.sqrt(DHEAD)

    with tile.TileContext(nc) as tc:
        import contextlib
        with contextlib.ExitStack() as ctx:
            const = ctx.enter_context(tc.tile_pool(name="const", bufs=1))
            dram = ctx.enter_context(tc.tile_pool(name="dram", bufs=1,
                                                  space="DRAM"))

            # ============ constants ============
            ident = const.tile([P, P], BF16)
            make_identity(nc, ident[:])
            ones_bf = const.tile([P, 1], BF16)
            nc.vector.memset(ones_bf[:], 1.0)
            # causal masks for the 4 diagonal key-tile offsets within a chunk:
            # cmask[j]: keep (=1.0) where query_col - p - 128*j >= 0 else 0
            cmask = []
            for j in range(SC // P):
                mk = const.tile([P, SC], BF16, name=f"cmask{j}")
                nc.vector.memset(mk[:], 1.0)
                nc.gpsimd.affine_select(mk[:], mk[:], pattern=[[1, SC]],
                                        base=-j * P, channel_multiplier=-1,
                                        compare_op=ALU.is_ge, fill=0.0)
                cmask.append(mk)

            # ============ DRAM scratch ============
            ag1_in = [dram.tile([BLK, SC], BF16, name=f"ag1_in{c}")
                      for c in range(NSC)]
            ag1_out = [dram.tile([N_CORES * BLK, SC], BF16, name=f"ag1_out{c}",
                                 addr_space="Shared") for c in range(NSC)]
            ag2_in = [dram.tile([BLK, SH], BF16, name=f"ag2_in{h}")
                      for h in range(2)]
            ag2_out = [dram.tile([N_CORES * BLK, SH], BF16, name=f"ag2_out{h}",
                                 addr_space="Shared") for h in range(2)]
            rs1_in = [dram.tile([HID, SC], BF16, name=f"rs1_in{c}")
                      for c in range(NSC)]
            rs1_out = [dram.tile([HID_SH, SC], BF16, name=f"rs1_out{c}")
                       for c in range(NSC)]
            rs2_in = [dram.tile([P * 8, SH], BF16, name=f"rs2_in{g}")
                      for g in range(8)]
            rs2_out = [dram.tile([P, SH], BF16, name=f"rs2_out{g}")
                      for g in range(8)]
            ag1_v = [t[:].rearrange("(c r) s -> c r s", r=BLK) for t in ag1_out]
            ag2_v = [t[:].rearrange("(c r) s -> c r s", r=BLK) for t in ag2_out]

            # ================== rope tables + era-A pools ==================
            era_a = ctx.enter_context(contextlib.ExitStack())
            a2a = era_a.enter_context(tc.tile_pool(name="a2a", bufs=1))
            a2b = era_a.enter_context(tc.tile_pool(name="a2b", bufs=1))
            pA1 = era_a.enter_context(tc.tile_pool(name="poolA1", bufs=1))

            # long-lived attn-era tiles
            xown = [a2a.tile([P, NT_HSH, SC], BF16, name=f"xown{c}")
                    for c in range(NSC)]
            # work tags for phase1 + r2
            wr = a2b

            if True:
                cos2 = pA1.tile([P, S], BF16, name="cos2")
                sin_neg = pA1.tile([P, S], BF16, name="sin_neg")
                s1b = pA1.tile([P, S], BF16, name="s1b")
                wqkv_sb = [pA1.tile([P, NT_HID, P], BF16, name=f"wqsb{j}")
                           for j in range(NJ)]
                wo_sb = pA1.tile([P, HQ, NT_HID, P], BF16, name="wosb")
                kT = pA1.tile([P, S], BF16, name="kT")
                vT = pA1.tile([P, S], BF16, name="vT")
                qc = [[pA1.tile([P, SC], BF16, name=f"qc{c}_{j}")
                       for j in range(HQ)] for c in range(NSC)]

                # weight loads first (scalar queue; independent of everything)
                for j in range(NJ):
                    nc.scalar.dma_start(wqkv_sb[j][:], wqkv[j])
                nc.scalar.dma_start(
                    wo_sb[:], wo[:].rearrange("p (a m c) -> p a m c",
                                              a=HQ, m=NT_HID))

                # ---- rope tables (scoped scratch, column-chunked) ----
                with tc.tile_pool(name="tbl", bufs=1) as tbl:
                    iot = tbl.tile([64, 1], I32)
                    nc.gpsimd.iota(iot[:], pattern=[[1, 1]], base=0,
                                   channel_multiplier=1)
                    iotf = tbl.tile([64, 1], F32)
                    nc.vector.tensor_copy(iotf[:], iot[:])
                    invf = tbl.tile([64, 1], F32)
                    nc.scalar.activation(invf[:], iotf[:], AF.Exp,
                                         scale=-math.log(THETA) / 64.0)
                    invf2pi = tbl.tile([64, 1], F32)
                    nc.scalar.activation(invf2pi[:], invf[:], AF.Copy,
                                         scale=1.0 / TWO_PI)

                    def range_reduce_sin(dst_bf, t_ap, negate=False):
                        # dst = sin(2*pi*t) via two-stage round-and-subtract
                        n1 = tbl.tile([64, SC], I32, tag="rri", bufs=2)
                        nc.vector.tensor_copy(n1[:], t_ap)
                        n1f = tbl.tile([64, SC], F32, tag="rrf", bufs=2)
                        nc.vector.tensor_copy(n1f[:], n1[:])
                        f1 = tbl.tile([64, SC], F32, tag="rrg", bufs=2)
                        nc.vector.tensor_tensor(f1[:], t_ap, n1f[:],
                                                ALU.subtract)
                        n2 = tbl.tile([64, SC], I32, tag="rri", bufs=2)
                        nc.vector.tensor_copy(n2[:], f1[:])
                        n2f = tbl.tile([64, SC], F32, tag="rrf", bufs=2)
                        nc.vector.tensor_copy(n2f[:], n2[:])
                        f2 = tbl.tile([64, SC], F32, tag="rrg", bufs=2)
                        nc.vector.tensor_tensor(f2[:], f1[:], n2f[:],
                                                ALU.subtract)
                        nc.scalar.activation(dst_bf, f2[:], AF.Sin,
                                             scale=-TWO_PI if negate else TWO_PI)

                    for c4 in range(NSC):
                        cols = slice(c4 * SC, (c4 + 1) * SC)
                        posi = tbl.tile([1, SC], I32, tag="posi", bufs=1)
                        nc.sync.dma_start(posi[:], pos_in[:, cols])
                        posf = tbl.tile([1, SC], F32, tag="posf", bufs=1)
                        nc.vector.tensor_copy(posf[:], posi[:])
                        posb = tbl.tile([64, SC], F32, tag="posb", bufs=1)
                        nc.gpsimd.partition_broadcast(posb[:], posf[:])
                        tfrac = tbl.tile([64, SC], F32, tag="tfr", bufs=1)
                        nc.scalar.activation(tfrac[:], posb[:], AF.Copy,
                                             scale=invf2pi[:])
                        sinb = tbl.tile([64, SC], BF16, tag="sb", bufs=1)
                        sinnb = tbl.tile([64, SC], BF16, tag="snb", bufs=1)
                        range_reduce_sin(sinb[:], tfrac[:])
                        range_reduce_sin(sinnb[:], tfrac[:], negate=True)
                        tfrac2 = tbl.tile([64, SC], F32, tag="tfr2", bufs=1)
                        nc.scalar.activation(tfrac2[:], tfrac[:], AF.Copy,
                                             bias=0.25)
                        cosb = tbl.tile([64, SC], BF16, tag="cb", bufs=1)
                        range_reduce_sin(cosb[:], tfrac2[:])
                        nc.sync.dma_start(cos2[:64, cols], cosb[:])
                        nc.sync.dma_start(cos2[64:, cols], cosb[:])
                        nc.sync.dma_start(sin_neg[:64, cols], sinnb[:])
                        nc.sync.dma_start(sin_neg[64:, cols], sinb[:])

                # ---- phase 1: x = h + r per chunk; ssq row; AG1(c) ----
                for c in range(NSC):
                    cs = slice(c * SC, (c + 1) * SC)
                    ssq_acc = wr.tile([1, SC], F32, tag="ssqrow", bufs=2,
                                      name=f"ssq1_{c}")
                    for i in range(NT_HSH):
                        a = wr.tile([P, SC], F32, tag="p1a", bufs=2)
                        b = wr.tile([P, SC], F32, tag="p1b", bufs=2)
                        nc.sync.dma_start(a[:], hT[i * P:(i + 1) * P, cs])
                        nc.sync.dma_start(b[:], rT[i * P:(i + 1) * P, cs])
                        xt = wr.tile([P, SC], F32, tag="p1x", bufs=3)
                        nc.vector.tensor_tensor(xt[:], a[:], b[:], ALU.add)
                        nc.vector.tensor_copy(xown[c][:, i, :], xt[:])
                        nc.sync.dma_start(ag1_in[c][i * P:(i + 1) * P, :],
                                          xown[c][:, i, :])
                        sq = wr.tile([P, SC], F32, tag="p1s", bufs=2)
                        nc.scalar.activation(sq[:], xt[:], AF.Square)
                        red = wr.tile([P, SC], F32, tag="p1r", bufs=2)
                        nc.gpsimd.partition_all_reduce(
                            red[:], sq[:], channels=P, reduce_op=ReduceOp.add)
                        if i == 0:
                            nc.vector.tensor_copy(ssq_acc[:], red[:1, :])
                        else:
                            nc.vector.tensor_tensor(ssq_acc[:], ssq_acc[:],
                                                    red[:1, :], ALU.add)
                    ssqb = wr.tile([1, SC], BF16, tag="ssqb", bufs=2)
                    nc.vector.tensor_copy(ssqb[:], ssq_acc[:])
                    nc.sync.dma_start(ag1_in[c][HID_SH:HID_SH + 1, :], ssqb[:])
                    nc.gpsimd.collective_compute(
                        "AllGather", ALU.bypass, replica_groups=RG,
                        ins=[ag1_in[c][:].opt()], outs=[ag1_out[c][:].opt()])

                # ================== era Q: qkv all chunks ==================
                with tc.tile_pool(name="workQ", bufs=1) as wq, \
                     tc.tile_pool(name="psQ", bufs=1, space="PSUM") as psQ:

                    def emit_s1(c):
                        cs = slice(c * SC, (c + 1) * SC)
                        srb = wq.tile([8, SC], BF16, tag="srb", bufs=2)
                        nc.gpsimd.dma_start(srb[:], ag1_v[c][:, HID_SH, :])
                        srf = wq.tile([8, SC], F32, tag="srf", bufs=2)
                        nc.vector.tensor_copy(srf[:], srb[:])
                        ssum = wq.tile([8, SC], F32, tag="ssum", bufs=2)
                        nc.gpsimd.partition_all_reduce(
                            ssum[:], srf[:], channels=8,
                            reduce_op=ReduceOp.add)
                        var = wq.tile([1, SC], F32, tag="var", bufs=2)
                        nc.scalar.activation(var[:], ssum[:1, :], AF.Copy,
                                             scale=1.0 / HID, bias=EPS)
                        nc.vector.reciprocal(var[:], var[:])
                        varb = wq.tile([1, SC], BF16, tag="varb", bufs=2)
                        nc.scalar.activation(varb[:], var[:], AF.Sqrt)
                        nc.gpsimd.partition_broadcast(s1b[:, cs], varb[:])

                    def g_qkv(c):
                        cs = slice(c * SC, (c + 1) * SC)
                        emit_s1(c)
                        ps = [psQ.tile([P, SC], F32, tag="qkvps", bufs=6,
                                       name=f"qps{c}_{j}") for j in range(NJ)]
                        for cb in range(N_CORES):
                            hgt = wq.tile([P, NT_HSH, SC], BF16, tag="hgr",
                                          bufs=3, name=f"hg{c}_{cb}")
                            nc.gpsimd.dma_start(
                                hgt[:], ag1_v[c][cb, :HID_SH, :].rearrange(
                                    "(t p) s -> p t s", p=P))
                            for t in range(NT_HSH):
                                kk = cb * NT_HSH + t
                                for j in range(NJ):
                                    nc.tensor.matmul(
                                        ps[j][:], wqkv_sb[j][:, kk, :],
                                        hgt[:, t, :], start=(kk == 0),
                                        stop=(kk == NT_HID - 1))
                                yield
                        for j in range(NJ):
                            if j < HQ:
                                dst = qc[c][j][:]
                            elif j == HQ:
                                dst = kT[:, cs]
                            else:
                                dst = vT[:, cs]
                            nc.vector.tensor_tensor(dst, ps[j][:], s1b[:, cs],
                                                    ALU.mult)
                        yield
                        # rope on q tiles + k (in place)
                        for j in range(HQ + 1):
                            tv = qc[c][j][:] if j < HQ else kT[:, cs]
                            swp = wq.tile([P, SC], BF16, tag="swp", bufs=2)
                            nc.sync.dma_start(swp[:64, :], tv[64:, :])
                            nc.sync.dma_start(swp[64:, :], tv[:64, :])
                            m1 = wq.tile([P, SC], BF16, tag="m1", bufs=2)
                            nc.vector.tensor_tensor(m1[:], tv, cos2[:, cs],
                                                    ALU.mult)
                            nc.vector.tensor_tensor(swp[:], swp[:],
                                                    sin_neg[:, cs], ALU.mult)
                            nc.vector.tensor_tensor(tv, m1[:], swp[:], ALU.add)
                            yield
                        # v transpose in place (block-transposed v)
                        for t in range(c * (SC // P), (c + 1) * (SC // P)):
                            pst = psQ.tile([P, P], BF16, tag="tp", bufs=2)
                            nc.tensor.transpose(pst[:], vT[:, t * P:(t + 1) * P],
                                                ident[:])
                            nc.vector.tensor_copy(vT[:, t * P:(t + 1) * P],
                                                  pst[:])
                        yield

                    for c in range(NSC):
                        for _ in g_qkv(c):
                            pass

                # ================== era A: attention ==================
                with tc.tile_pool(name="workA", bufs=1) as wa, \
                     tc.tile_pool(name="psA", bufs=1, space="PSUM") as psA:

                    atl = {}

                    def g_attn(c, look):
                        nsk = (c + 1) * (SC // P)
                        for h in range(HQ):
                            pvp = psA.tile([P, SC], F32, tag="pv", bufs=2,
                                           name=f"pv{c}_{h}")
                            rsp = psA.tile([1, SC], F32, tag="rs", bufs=2,
                                           name=f"rs{c}_{h}")
                            exs = {}

                            def scores(skt, h=h, c=c, exs=exs):
                                diag = skt - 4 * c
                                scp = psA.tile([P, SC], F32, tag="sc", bufs=4,
                                               name=f"sc{c}_{h}_{skt}")
                                nc.tensor.matmul(scp[:],
                                                 kT[:, skt * P:(skt + 1) * P],
                                                 qc[c][h][:],
                                                 start=True, stop=True)
                                ex = wa.tile([P, SC], BF16, tag="ex", bufs=6,
                                             name=f"ex{c}_{h}_{skt}")
                                nc.scalar.activation(ex[:], scp[:],
                                                     AF.Exp, scale=inv_sqrt_d)
                                if diag >= 0:
                                    nc.vector.tensor_tensor(
                                        ex[:], ex[:], cmask[diag][:], ALU.mult)
                                exs[skt] = ex

                            def accum(skt, h=h, c=c, exs=exs, pvp=pvp,
                                      rsp=rsp, nsk=nsk):
                                ex = exs.pop(skt)
                                nc.tensor.matmul(rsp[:], ones_bf[:],
                                                 ex[:],
                                                 start=(skt == 0),
                                                 stop=(skt == nsk - 1))
                                nc.tensor.matmul(pvp[:],
                                                 vT[:, skt * P:(skt + 1) * P],
                                                 ex[:],
                                                 start=(skt == 0),
                                                 stop=(skt == nsk - 1))

                            if look:
                                scores(0)
                                for skt in range(nsk):
                                    if skt + 1 < nsk:
                                        scores(skt + 1)
                                    accum(skt)
                                    yield
                            else:
                                for skt in range(nsk):
                                    scores(skt)
                                    accum(skt)
                                    yield
                            rcp = wa.tile([1, SC], F32, tag="rcp", bufs=2)
                            nc.vector.reciprocal(rcp[:], rsp[:])
                            rcpb = wa.tile([P, SC], F32, tag="rcpb", bufs=2)
                            nc.gpsimd.partition_broadcast(rcpb[:], rcp[:])
                            att = wa.tile([P, SC], BF16, tag="at", bufs=16,
                                          name=f"at{c}_{h}")
                            atl[(c, h)] = att
                            nc.vector.tensor_tensor(att[:], pvp[:], rcpb[:],
                                                    ALU.mult)
                            yield

                    def g_o(c):
                        for m in range(NT_HID):
                            ps = psA.tile([P, SC], F32, tag="sc", bufs=4,
                                          name=f"o{c}_{m}")
                            for a in range(HQ):
                                nc.tensor.matmul(ps[:], wo_sb[:, a, m, :],
                                                 atl[(c, a)][:],
                                                 start=(a == 0),
                                                 stop=(a == HQ - 1))
                            ev = wa.tile([P, SC], BF16, tag="oev", bufs=3)
                            nc.vector.tensor_copy(ev[:], ps[:])
                            nc.sync.dma_start(rs1_in[c][m * P:(m + 1) * P, :],
                                              ev[:])
                            yield
                        nc.gpsimd.collective_compute(
                            "ReduceScatter", ALU.add, replica_groups=RG,
                            ins=[rs1_in[c][:].opt()], outs=[rs1_out[c][:].opt()])
                        yield

                    def g_r2(c):
                        # residual2 = o_proj partial-sum + x (Pool/scalar/sync
                        # engines only — must not HOL-block PE/DVE streams)
                        cs = slice(c * SC, (c + 1) * SC)
                        hh = c // 2
                        hcs = slice((c % 2) * SC, (c % 2 + 1) * SC)
                        ssq_acc = wr.tile([1, SC], F32, tag="ssqrow", bufs=2,
                                          name=f"ssq2_{c}")
                        for i in range(NT_HSH):
                            o_i = wr.tile([P, SC], BF16, tag="r2o", bufs=2)
                            nc.gpsimd.dma_start(
                                o_i[:], rs1_out[c][i * P:(i + 1) * P, :])
                            r2t = wr.tile([P, SC], F32, tag="p1x", bufs=3)
                            nc.gpsimd.tensor_tensor(r2t[:], o_i[:],
                                                    xown[c][:, i, :], ALU.add)
                            nc.sync.dma_start(out_res2[i * P:(i + 1) * P, cs],
                                              r2t[:])
                            r2b = wr.tile([P, SC], BF16, tag="r2b", bufs=2)
                            nc.scalar.activation(r2b[:], r2t[:], AF.Copy)
                            nc.sync.dma_start(
                                ag2_in[hh][i * P:(i + 1) * P, hcs], r2b[:])
                            sq = wr.tile([P, SC], F32, tag="p1s", bufs=2)
                            nc.scalar.activation(sq[:], r2t[:], AF.Square)
                            red = wr.tile([P, SC], F32, tag="p1r", bufs=2)
                            nc.gpsimd.partition_all_reduce(
                                red[:], sq[:], channels=P,
                                reduce_op=ReduceOp.add)
                            if i == 0:
                                nc.scalar.activation(ssq_acc[:], red[:1, :],
                                                     AF.Copy)
                            else:
                                nc.gpsimd.tensor_tensor(ssq_acc[:], ssq_acc[:],
                                                        red[:1, :], ALU.add)
                            yield
                        sb = wr.tile([1, SC], BF16, tag="ssqb", bufs=2)
                        nc.scalar.activation(sb[:], ssq_acc[:], AF.Copy)
                        nc.sync.dma_start(
                            ag2_in[hh][HID_SH:HID_SH + 1, hcs], sb[:])
                        if c % 2 == 1:
                            nc.gpsimd.collective_compute(
                                "AllGather", ALU.bypass, replica_groups=RG,
                                ins=[ag2_in[hh][:].opt()],
                                outs=[ag2_out[hh][:].opt()])
                        yield

                    # era 5: first chunk pair, self-pipelined scores
                    weave(g_attn(0, True), g_attn(1, True))
                    # era 6: second chunk pair + o(0,1) as PE filler
                    weave(g_attn(2, False), g_attn(3, False),
                          (g_o(0), 2), (g_o(1), 2))
                    # era 6b: residual2(0,1) (non-PE; RS1(0/1) already fired),
                    # then o(2,3), then residual2(2,3) behind their RS1s
                    weave(g_r2(0), g_r2(1))
                    weave(g_o(2), g_o(3))
                    weave(g_r2(2), g_r2(3))

            # close attention-era pools before the MLP era
            era_a.close()

            # ============== era M: MLP ==============
            if True:
                    with tc.tile_pool(name="workM", bufs=1) as wm, \
                         tc.tile_pool(name="psM", bufs=1, space="PSUM") as psM:

                        s2b = wm.tile([P, S], F32, name="s2b")
                        uT = wm.tile([P, NT_INT, S], BF16, name="uT")

                        def emit_s2(h):
                            tok = slice(h * SH, (h + 1) * SH)
                            srb = wm.tile([8, SH], BF16, tag="srb2", bufs=2)
                            nc.gpsimd.dma_start(srb[:], ag2_v[h][:, HID_SH, :])
                            srf = wm.tile([8, SH], F32, tag="srf2", bufs=2)
                            nc.vector.tensor_copy(srf[:], srb[:])
                            ssum = wm.tile([8, SH], F32, tag="ssum2", bufs=2)
                            nc.gpsimd.partition_all_reduce(
                                ssum[:], srf[:], channels=8,
                                reduce_op=ReduceOp.add)
                            var = wm.tile([1, SH], F32, tag="var2", bufs=2)
                            nc.scalar.activation(var[:], ssum[:1, :], AF.Copy,
                                                 scale=1.0 / HID, bias=EPS)
                            nc.vector.reciprocal(var[:], var[:])  # = scale^2
                            nc.gpsimd.partition_broadcast(s2b[:, tok], var[:])

                        def g_up(h):
                            tok = slice(h * SH, (h + 1) * SH)
                            emit_s2(h)
                            h2g = wm.tile([P, NT_HID, SH], BF16, tag="h2g",
                                          bufs=1, name=f"h2g{h}")
                            for cb in range(N_CORES):
                                nc.scalar.dma_start(
                                    h2g[:, cb * NT_HSH:(cb + 1) * NT_HSH, :],
                                    ag2_v[h][cb, :HID_SH, :].rearrange(
                                        "(t p) s -> p t s", p=P))
                            for it in range(NT_INT):
                                wt = wm.tile([P, NT_HID, P], BF16, tag="wup",
                                             bufs=2, name=f"wup{h}_{it}")
                                nc.scalar.dma_start(wt[:], wup[it])
                                ps = [psM.tile([P, SC], F32, tag="up", bufs=4,
                                               name=f"up{h}_{it}_{t2}")
                                      for t2 in range(2)]
                                for k in range(NT_HID):
                                    for t2 in range(2):
                                        nc.tensor.matmul(
                                            ps[t2][:], wt[:, k, :],
                                            h2g[:, k, t2 * SC:(t2 + 1) * SC],
                                            start=(k == 0),
                                            stop=(k == NT_HID - 1))
                                    if k % 8 == 7:
                                        yield
                                for t2 in range(2):
                                    tkc = slice(h * SH + t2 * SC,
                                                h * SH + (t2 + 1) * SC)
                                    rl = wm.tile([P, SC], F32, tag="rl",
                                                 bufs=2)
                                    nc.scalar.activation(rl[:], ps[t2][:],
                                                         AF.Relu)
                                    nc.vector.tensor_tensor(uT[:, it, tkc],
                                                            rl[:], rl[:],
                                                            ALU.mult)
                                yield

                        def g_down(th):
                            tok0 = th * SH
                            for q in range(4):
                                for m8 in range(8):
                                    m = q * 8 + m8
                                    wt = wm.tile([P, NT_INT, P], BF16,
                                                 tag="wdn", bufs=3,
                                                 name=f"wdn{th}_{m}")
                                    nc.scalar.dma_start(wt[:], wdn[m])
                                    for t2 in range(2):
                                        tok = slice(tok0 + t2 * SC,
                                                    tok0 + (t2 + 1) * SC)
                                        ps = psM.tile([P, SC], F32, tag="dn",
                                                      bufs=2,
                                                      name=f"dn{th}_{m}_{t2}")
                                        for it in range(NT_INT):
                                            nc.tensor.matmul(
                                                ps[:], wt[:, it, :],
                                                uT[:, it, tok],
                                                start=(it == 0),
                                                stop=(it == NT_INT - 1))
                                        ev = wm.tile([P, SC], BF16, tag="dnev",
                                                     bufs=3)
                                        nc.vector.tensor_tensor(
                                            ev[:], ps[:], s2b[:, tok],
                                            ALU.mult)
                                        nc.sync.dma_start(
                                            rs2_in[q * 2 + th][
                                                m8 * P:(m8 + 1) * P,
                                                t2 * SC:(t2 + 1) * SC], ev[:])
                                        yield
                                g = q * 2 + th
                                nc.gpsimd.collective_compute(
                                    "ReduceScatter", ALU.add,
                                    replica_groups=RG,
                                    ins=[rs2_in[g][:].opt()],
                                    outs=[rs2_out[g][:].opt()])
                                nc.gpsimd.dma_start(
                                    out_mlp[q * P:(q + 1) * P,
                                            tok0:tok0 + SH], rs2_out[g][:])
                                yield

                        # era 7: up(half0)
                        weave(g_up(0))
                        # era 8: up(half1) + down(token half0)
                        weave(g_up(1), g_down(0))
                        # era 9: down(token half1)
                        weave(g_down(1))

    nc.compile()
    return nc


def shard_inputs(positions, hidden_states, residual, qkv_w, o_w, up_w, down_w,
                 ln1_w, ln2_w):
    import ml_dtypes
    BF = ml_dtypes.bfloat16
    hTf = np.ascontiguousarray(np.asarray(hidden_states, np.float32)
                               .reshape(S, HID).T)
    rTf = np.ascontiguousarray(np.asarray(residual, np.float32)
                               .reshape(S, HID).T)
    pos = np.ascontiguousarray(np.asarray(positions).reshape(1, S))
    ln1 = np.asarray(ln1_w, np.float32)
    ln2 = np.asarray(ln2_w, np.float32)
    qkv_w = np.asarray(qkv_w, np.float32) * ln1[:, None]
    up_w = np.asarray(up_w, np.float32) * ln2[:, None]
    o_w = np.asarray(o_w, np.float32)
    down_w = np.asarray(down_w, np.float32)
    q_size = N_HEADS * DHEAD
    kv = N_KV * DHEAD
    in_maps = []
    for c in range(N_CORES):
        wq_c = np.concatenate([
            qkv_w[:, c * HQ * DHEAD:(c + 1) * HQ * DHEAD],
            qkv_w[:, q_size + c * DHEAD:q_size + (c + 1) * DHEAD],
            qkv_w[:, q_size + kv + c * DHEAD:q_size + kv + (c + 1) * DHEAD],
        ], axis=1)                                   # [4096, 768]
        wq_p = wq_c.reshape(NT_HID, P, NJ, P).transpose(2, 1, 0, 3) \
            .reshape(NJ, P, NT_HID * P)
        wo_c = o_w[c * HQ * DHEAD:(c + 1) * HQ * DHEAD, :]   # [512, 4096]
        wo_p = wo_c.reshape(HQ, P, NT_HID, P).transpose(1, 0, 2, 3) \
            .reshape(P, HQ * NT_HID * P)
        wup_c = up_w[:, c * INT_SH:(c + 1) * INT_SH]         # [4096, 2048]
        wup_p = wup_c.reshape(NT_HID, P, NT_INT, P).transpose(2, 1, 0, 3) \
            .reshape(NT_INT, P, NT_HID * P)
        wdn_c = down_w[c * INT_SH:(c + 1) * INT_SH, :]       # [2048, 4096]
        wdn_p = wdn_c.reshape(NT_INT, P, NT_HID, P).transpose(2, 1, 0, 3) \
            .reshape(NT_HID, P, NT_INT * P)
        in_maps.append({
            "hT": np.ascontiguousarray(hTf[c * HID_SH:(c + 1) * HID_SH]),
            "rT": np.ascontiguousarray(rTf[c * HID_SH:(c + 1) * HID_SH]),
            "positions": pos,
            "wqkv": np.ascontiguousarray(wq_p.astype(BF)),
            "wo": np.ascontiguousarray(wo_p.astype(BF)),
            "wup": np.ascontiguousarray(wup_p.astype(BF)),
            "wdn": np.ascontiguousarray(wdn_p.astype(BF)),
        })
    return in_maps


_CACHE = {}


def kernel(**inputs):
    from concourse.bass_utils import run_bass_kernel_spmd
    if "nc" not in _CACHE:
        _CACHE["nc"] = build_graph()
    nc = _CACHE["nc"]
    in_maps = shard_inputs(**{k: np.asarray(v) for k, v in inputs.items()})
    res = run_bass_kernel_spmd(nc, in_maps, core_ids=list(range(N_CORES)),
                               trace=False)
    res2T = np.concatenate([res.results[c]["res2T"] for c in range(N_CORES)],
                           axis=0)
    mlpT = np.empty((HID, S), np.float32)
    for c in range(N_CORES):
        mt = res.results[c]["mlpT"]
        for q in range(4):
            mlpT[q * 1024 + c * P:q * 1024 + c * P + P] = \
                mt[q * P:(q + 1) * P]
    mlp_out = np.ascontiguousarray(mlpT.T).reshape(1, S, HID)
    residual2 = np.ascontiguousarray(res2T.T).reshape(1, S, HID)
    return mlp_out, residual2


# revision 20
# speedup vs baseline: 1.1788x; 1.0483x over previous
"""Arcee decoder layer on 8 TRN2 NeuronCores — tensor-parallel Bass kernel v2.

Sharding (8-way TP, transposed activation layout [hidden, seq] on device):
  - core c owns: q heads 4c..4c+3 + kv head c (GQA group), residual-stream
    rows 512c..512c+512, intermediate cols 2048c..2048c+2048.
  - Weights are cast to bf16 and packed into SBUF-tile layout on the host
    (ln1 folded into qkv weights, ln2 folded into up weights).
  - RMSNorm trick: the un-normalized residual stream is AllGathered (bf16,
    per-chunk) with each core's partial sum-of-squares embedded as an extra
    row; every core derives the per-token rsqrt scale locally; the scale is
    applied at qkv eviction (attn) / folded into the down eviction (mlp).
  - o_proj emits transposed partials [4096,S-chunk] -> bf16 ReduceScatter
    per chunk; down_proj -> 8 ReduceScatters of [1024, 1024].
  - Emission order is software-pipelined: qkv for all 4 seq chunks first
    (PE-dense), then attention chunk pairs woven with o_proj / residual2 /
    up-proj streams so the PE never head-of-line blocks on collectives.
"""
import sys

sys.path.insert(0, "/opt/trn_rl_repo")

import math
import numpy as np

import concourse.bass as bass
import concourse.mybir as mybir
import concourse.tile as tile
from concourse import bacc
from concourse.bass_isa import ReduceOp
from concourse.masks import make_identity

F32 = mybir.dt.float32
BF16 = mybir.dt.bfloat16
I32 = mybir.dt.int32
AF = mybir.ActivationFunctionType
ALU = mybir.AluOpType

N_CORES = 8
S = 2048
HID = 4096
N_HEADS = 32
N_KV = 8
DHEAD = 128
INTER = 16384
EPS = 1e-5
THETA = 10000.0

HQ = N_HEADS // N_CORES          # 4 q heads per core
HID_SH = HID // N_CORES          # 512 residual rows per core
INT_SH = INTER // N_CORES        # 2048 intermediate per core
NJ = HQ + 2                      # qkv col tiles per core (4q + k + v)
P = 128
SC = 512                         # seq chunk
NSC = S // SC                    # 4
SH = S // 2                      # token half
NT_HID = HID // P                # 32
NT_HSH = HID_SH // P             # 4
NT_INT = INT_SH // P             # 16
BLK = HID_SH + 8                 # AG block rows: 512 payload + ssq row + pad
TWO_PI = 2.0 * math.pi

DIAG_TRIM = True                 # restrict diagonal score tiles to valid cols


def weave(*streams):
    """Round-robin generators: (gen, weight) or gen (weight 1)."""
    live = []
    for s in streams:
        if isinstance(s, tuple):
            live.append([s[0], s[1]])
        else:
            live.append([s, 1])
    while live:
        for ent in list(live):
            g, w = ent
            for _ in range(w):
                try:
                    next(g)
                except StopIteration:
                    live.remove(ent)
                    break


def build_graph():
    nc = bacc.Bacc(None, target_bir_lowering=False, debug=False)

    hT = nc.declare_dram_parameter("hT", [HID_SH, S], F32, isOutput=False)
    rT = nc.declare_dram_parameter("rT", [HID_SH, S], F32, isOutput=False)
    pos_in = nc.declare_dram_parameter("positions", [1, S], I32, isOutput=False)
    wqkv = nc.declare_dram_parameter("wqkv", [NJ, P, NT_HID * P], BF16,
                                     isOutput=False)
    wo = nc.declare_dram_parameter("wo", [P, HQ * NT_HID * P], BF16,
                                   isOutput=False)
    wup = nc.declare_dram_parameter("wup", [NT_INT, P, NT_HID * P], BF16,
                                    isOutput=False)
    wdn = nc.declare_dram_parameter("wdn", [NT_HID, P, NT_INT * P], BF16,
                                    isOutput=False)
    out_res2 = nc.declare_dram_parameter("res2T", [HID_SH, S], F32,
                                         isOutput=True)
    out_mlp = nc.declare_dram_parameter("mlpT", [HID_SH, S], F32,
                                        isOutput=True)

    RG = [list(range(N_CORES))]
    inv_sqrt_d = 1.0 / math.sqrt(DHEAD)

    with tile.TileContext(nc) as tc:
        import contextlib
        with contextlib.ExitStack() as ctx:
            const = ctx.enter_context(tc.tile_pool(name="const", bufs=1))
            dram = ctx.enter_context(tc.tile_pool(name="dram", bufs=1,
                                                  space="DRAM"))

            # ============ constants ============
            ident = const.tile([P, P], BF16)
            make_identity(nc, ident[:])
            ones_bf = const.tile([P, 1], BF16)
            nc.vector.memset(ones_bf[:], 1.0)
            # causal masks for the 4 diagonal key-tile offsets within a chunk:
            # cmask[j]: keep (=1.0) where query_col - p - 128*j >= 0 else 0
            cmask = []
            for j in range(SC // P):
                mk = const.tile([P, SC], BF16, name=f"cmask{j}")
                nc.vector.memset(mk[:], 1.0)
                nc.gpsimd.affine_select(mk[:], mk[:], pattern=[[1, SC]],
                                        base=-j * P, channel_multiplier=-1,
                                        compare_op=ALU.is_ge, fill=0.0)
                cmask.append(mk)

            # ============ DRAM scratch ============
            ag1_in = [dram.tile([BLK, SC], BF16, name=f"ag1_in{c}")
                      for c in range(NSC)]
            ag1_out = [dram.tile([N_CORES * BLK, SC], BF16, name=f"ag1_out{c}",
                                 addr_space="Shared") for c in range(NSC)]
            ag2_in = [dram.tile([BLK, SH], BF16, name=f"ag2_in{h}")
                      for h in range(2)]
            ag2_out = [dram.tile([N_CORES * BLK, SH], BF16, name=f"ag2_out{h}",
                                 addr_space="Shared") for h in range(2)]
            rs1_in = [dram.tile([HID, SC], BF16, name=f"rs1_in{c}")
                      for c in range(NSC)]
            rs1_out = [dram.tile([HID_SH, SC], BF16, name=f"rs1_out{c}")
                       for c in range(NSC)]
            rs2_in = [dram.tile([P * 8, SH], BF16, name=f"rs2_in{g}")
                      for g in range(8)]
            rs2_out = [dram.tile([P, SH], BF16, name=f"rs2_out{g}")
                      for g in range(8)]
            ag1_v = [t[:].rearrange("(c r) s -> c r s", r=BLK) for t in ag1_out]
            ag2_v = [t[:].rearrange("(c r) s -> c r s", r=BLK) for t in ag2_out]

            # ================== rope tables + era-A pools ==================
            a2a = ctx.enter_context(tc.tile_pool(name="a2a", bufs=1))
            a2b = ctx.enter_context(tc.tile_pool(name="a2b", bufs=1))
            era_a = ctx.enter_context(contextlib.ExitStack())
            pA1 = era_a.enter_context(tc.tile_pool(name="poolA1", bufs=1))

            # long-lived attn-era tiles
            xown = [a2a.tile([P, NT_HSH, SC], BF16, name=f"xown{c}")
                    for c in range(NSC)]
            # work tags for phase1 + r2
            wr = a2b

            if True:
                cos2 = pA1.tile([P, S], BF16, name="cos2")
                sin_neg = pA1.tile([P, S], BF16, name="sin_neg")
                s1b = pA1.tile([P, S], BF16, name="s1b")
                wqkv_sb = [pA1.tile([P, NT_HID, P], BF16, name=f"wqsb{j}")
                           for j in range(NJ)]
                wo_sb = pA1.tile([P, HQ, NT_HID, P], BF16, name="wosb")
                kT = pA1.tile([P, S], BF16, name="kT")
                vT = pA1.tile([P, S], BF16, name="vT")
                qc = [[pA1.tile([P, SC], BF16, name=f"qc{c}_{j}")
                       for j in range(HQ)] for c in range(NSC)]

                # weight loads first (scalar queue; independent of everything)
                for j in range(NJ):
                    nc.scalar.dma_start(wqkv_sb[j][:], wqkv[j])
                nc.scalar.dma_start(
                    wo_sb[:], wo[:].rearrange("p (a m c) -> p a m c",
                                              a=HQ, m=NT_HID))

                # ---- rope tables (scoped scratch, column-chunked) ----
                with tc.tile_pool(name="tbl", bufs=1) as tbl:
                    iot = tbl.tile([64, 1], I32)
                    nc.gpsimd.iota(iot[:], pattern=[[1, 1]], base=0,
                                   channel_multiplier=1)
                    iotf = tbl.tile([64, 1], F32)
                    nc.vector.tensor_copy(iotf[:], iot[:])
                    invf = tbl.tile([64, 1], F32)
                    nc.scalar.activation(invf[:], iotf[:], AF.Exp,
                                         scale=-math.log(THETA) / 64.0)
                    invf2pi = tbl.tile([64, 1], F32)
                    nc.scalar.activation(invf2pi[:], invf[:], AF.Copy,
                                         scale=1.0 / TWO_PI)

                    def range_reduce_sin(dst_bf, t_ap, negate=False):
                        # dst = sin(2*pi*t) via two-stage round-and-subtract
                        n1 = tbl.tile([64, SC], I32, tag="rri", bufs=2)
                        nc.vector.tensor_copy(n1[:], t_ap)
                        n1f = tbl.tile([64, SC], F32, tag="rrf", bufs=2)
                        nc.vector.tensor_copy(n1f[:], n1[:])
                        f1 = tbl.tile([64, SC], F32, tag="rrg", bufs=2)
                        nc.vector.tensor_tensor(f1[:], t_ap, n1f[:],
                                                ALU.subtract)
                        n2 = tbl.tile([64, SC], I32, tag="rri", bufs=2)
                        nc.vector.tensor_copy(n2[:], f1[:])
                        n2f = tbl.tile([64, SC], F32, tag="rrf", bufs=2)
                        nc.vector.tensor_copy(n2f[:], n2[:])
                        f2 = tbl.tile([64, SC], F32, tag="rrg", bufs=2)
                        nc.vector.tensor_tensor(f2[:], f1[:], n2f[:],
                                                ALU.subtract)
                        nc.scalar.activation(dst_bf, f2[:], AF.Sin,
                                             scale=-TWO_PI if negate else TWO_PI)

                    for c4 in range(NSC):
                        cols = slice(c4 * SC, (c4 + 1) * SC)
                        posi = tbl.tile([1, SC], I32, tag="posi", bufs=1)
                        nc.sync.dma_start(posi[:], pos_in[:, cols])
                        posf = tbl.tile([1, SC], F32, tag="posf", bufs=1)
                        nc.vector.tensor_copy(posf[:], posi[:])
                        posb = tbl.tile([64, SC], F32, tag="posb", bufs=1)
                        nc.gpsimd.partition_broadcast(posb[:], posf[:])
                        tfrac = tbl.tile([64, SC], F32, tag="tfr", bufs=1)
                        nc.scalar.activation(tfrac[:], posb[:], AF.Copy,
                                             scale=invf2pi[:])
                        sinb = tbl.tile([64, SC], BF16, tag="sb", bufs=1)
                        sinnb = tbl.tile([64, SC], BF16, tag="snb", bufs=1)
                        range_reduce_sin(sinb[:], tfrac[:])
                        range_reduce_sin(sinnb[:], tfrac[:], negate=True)
                        tfrac2 = tbl.tile([64, SC], F32, tag="tfr2", bufs=1)
                        nc.scalar.activation(tfrac2[:], tfrac[:], AF.Copy,
                                             bias=0.25)
                        cosb = tbl.tile([64, SC], BF16, tag="cb", bufs=1)
                        range_reduce_sin(cosb[:], tfrac2[:])
                        nc.sync.dma_start(cos2[:64, cols], cosb[:])
                        nc.sync.dma_start(cos2[64:, cols], cosb[:])
                        nc.sync.dma_start(sin_neg[:64, cols], sinnb[:])
                        nc.sync.dma_start(sin_neg[64:, cols], sinb[:])

                # ---- phase 1: x = h + r per chunk; ssq row; AG1(c) ----
                for c in range(NSC):
                    cs = slice(c * SC, (c + 1) * SC)
                    ssq_acc = wr.tile([1, SC], F32, tag="ssqrow", bufs=2,
                                      name=f"ssq1_{c}")
                    for i in range(NT_HSH):
                        a = wr.tile([P, SC], F32, tag="p1a", bufs=2)
                        b = wr.tile([P, SC], F32, tag="p1b", bufs=2)
                        nc.sync.dma_start(a[:], hT[i * P:(i + 1) * P, cs])
                        nc.sync.dma_start(b[:], rT[i * P:(i + 1) * P, cs])
                        xt = wr.tile([P, SC], F32, tag="p1x", bufs=3)
                        nc.vector.tensor_tensor(xt[:], a[:], b[:], ALU.add)
                        nc.vector.tensor_copy(xown[c][:, i, :], xt[:])
                        nc.sync.dma_start(ag1_in[c][i * P:(i + 1) * P, :],
                                          xown[c][:, i, :])
                        sq = wr.tile([P, SC], F32, tag="p1s", bufs=2)
                        nc.scalar.activation(sq[:], xt[:], AF.Square)
                        red = wr.tile([P, SC], F32, tag="p1r", bufs=2)
                        nc.gpsimd.partition_all_reduce(
                            red[:], sq[:], channels=P, reduce_op=ReduceOp.add)
                        if i == 0:
                            nc.vector.tensor_copy(ssq_acc[:], red[:1, :])
                        else:
                            nc.vector.tensor_tensor(ssq_acc[:], ssq_acc[:],
                                                    red[:1, :], ALU.add)
                    ssqb = wr.tile([1, SC], BF16, tag="ssqb", bufs=2)
                    nc.vector.tensor_copy(ssqb[:], ssq_acc[:])
                    nc.sync.dma_start(ag1_in[c][HID_SH:HID_SH + 1, :], ssqb[:])
                    nc.gpsimd.collective_compute(
                        "AllGather", ALU.bypass, replica_groups=RG,
                        ins=[ag1_in[c][:].opt()], outs=[ag1_out[c][:].opt()])

                # ================== era Q: qkv all chunks ==================
                with tc.tile_pool(name="workQ", bufs=1) as wq, \
                     tc.tile_pool(name="psQ", bufs=1, space="PSUM") as psQ:

                    def emit_s1(c):
                        cs = slice(c * SC, (c + 1) * SC)
                        srb = wq.tile([8, SC], BF16, tag="srb", bufs=2)
                        nc.gpsimd.dma_start(srb[:], ag1_v[c][:, HID_SH, :])
                        srf = wq.tile([8, SC], F32, tag="srf", bufs=2)
                        nc.vector.tensor_copy(srf[:], srb[:])
                        ssum = wq.tile([8, SC], F32, tag="ssum", bufs=2)
                        nc.gpsimd.partition_all_reduce(
                            ssum[:], srf[:], channels=8,
                            reduce_op=ReduceOp.add)
                        var = wq.tile([1, SC], F32, tag="var", bufs=2)
                        nc.scalar.activation(var[:], ssum[:1, :], AF.Copy,
                                             scale=1.0 / HID, bias=EPS)
                        nc.vector.reciprocal(var[:], var[:])
                        varb = wq.tile([1, SC], BF16, tag="varb", bufs=2)
                        nc.scalar.activation(varb[:], var[:], AF.Sqrt)
                        nc.gpsimd.partition_broadcast(s1b[:, cs], varb[:])

                    def g_qkv(c):
                        cs = slice(c * SC, (c + 1) * SC)
                        emit_s1(c)
                        ps = [psQ.tile([P, SC], F32, tag="qkvps", bufs=6,
                                       name=f"qps{c}_{j}") for j in range(NJ)]
                        for cb in range(N_CORES):
                            hgt = wq.tile([P, NT_HSH, SC], BF16, tag="hgr",
                                          bufs=3, name=f"hg{c}_{cb}")
                            nc.gpsimd.dma_start(
                                hgt[:], ag1_v[c][cb, :HID_SH, :].rearrange(
                                    "(t p) s -> p t s", p=P))
                            for t in range(NT_HSH):
                                kk = cb * NT_HSH + t
                                for j in range(NJ):
                                    nc.tensor.matmul(
                                        ps[j][:], wqkv_sb[j][:, kk, :],
                                        hgt[:, t, :], start=(kk == 0),
                                        stop=(kk == NT_HID - 1))
                                yield
                        for j in range(NJ):
                            if j < HQ:
                                dst = qc[c][j][:]
                            elif j == HQ:
                                dst = kT[:, cs]
                            else:
                                dst = vT[:, cs]
                            nc.vector.tensor_tensor(dst, ps[j][:], s1b[:, cs],
                                                    ALU.mult)
                        yield
                        # rope on q tiles + k (in place)
                        for j in range(HQ + 1):
                            tv = qc[c][j][:] if j < HQ else kT[:, cs]
                            swp = wq.tile([P, SC], BF16, tag="swp", bufs=2)
                            nc.sync.dma_start(swp[:64, :], tv[64:, :])
                            nc.sync.dma_start(swp[64:, :], tv[:64, :])
                            m1 = wq.tile([P, SC], BF16, tag="m1", bufs=2)
                            nc.vector.tensor_tensor(m1[:], tv, cos2[:, cs],
                                                    ALU.mult)
                            nc.vector.tensor_tensor(swp[:], swp[:],
                                                    sin_neg[:, cs], ALU.mult)
                            nc.vector.tensor_tensor(tv, m1[:], swp[:], ALU.add)
                            yield
                        # v transpose in place (block-transposed v)
                        for t in range(c * (SC // P), (c + 1) * (SC // P)):
                            pst = psQ.tile([P, P], BF16, tag="tp", bufs=2)
                            nc.tensor.transpose(pst[:], vT[:, t * P:(t + 1) * P],
                                                ident[:])
                            nc.vector.tensor_copy(vT[:, t * P:(t + 1) * P],
                                                  pst[:])
                        yield

                    for c in range(NSC):
                        for _ in g_qkv(c):
                            pass

                # ================== era A: attention ==================
                with tc.tile_pool(name="workA", bufs=1) as wa, \
                     tc.tile_pool(name="psA", bufs=1, space="PSUM") as psA:

                    atl = {}

                    def g_attn(c, look):
                        nsk = (c + 1) * (SC // P)
                        for h in range(HQ):
                            pvp = psA.tile([P, SC], F32, tag="pv", bufs=2,
                                           name=f"pv{c}_{h}")
                            rsp = psA.tile([1, SC], F32, tag="rs", bufs=2,
                                           name=f"rs{c}_{h}")
                            exs = {}

                            def scores(skt, h=h, c=c, exs=exs):
                                diag = skt - 4 * c
                                scp = psA.tile([P, SC], F32, tag="sc", bufs=4,
                                               name=f"sc{c}_{h}_{skt}")
                                nc.tensor.matmul(scp[:],
                                                 kT[:, skt * P:(skt + 1) * P],
                                                 qc[c][h][:],
                                                 start=True, stop=True)
                                ex = wa.tile([P, SC], BF16, tag="ex", bufs=6,
                                             name=f"ex{c}_{h}_{skt}")
                                nc.scalar.activation(ex[:], scp[:],
                                                     AF.Exp, scale=inv_sqrt_d)
                                if diag >= 0:
                                    nc.vector.tensor_tensor(
                                        ex[:], ex[:], cmask[diag][:], ALU.mult)
                                exs[skt] = ex

                            def accum(skt, h=h, c=c, exs=exs, pvp=pvp,
                                      rsp=rsp, nsk=nsk):
                                ex = exs.pop(skt)
                                nc.tensor.matmul(rsp[:], ones_bf[:],
                                                 ex[:],
                                                 start=(skt == 0),
                                                 stop=(skt == nsk - 1))
                                nc.tensor.matmul(pvp[:],
                                                 vT[:, skt * P:(skt + 1) * P],
                                                 ex[:],
                                                 start=(skt == 0),
                                                 stop=(skt == nsk - 1))

                            if look:
                                scores(0)
                                for skt in range(nsk):
                                    if skt + 1 < nsk:
                                        scores(skt + 1)
                                    accum(skt)
                                    yield
                            else:
                                for skt in range(nsk):
                                    scores(skt)
                                    accum(skt)
                                    yield
                            # free the pv accumulation bank immediately;
                            # normalize out-of-band so the next head's psum
                            # allocation never waits on the rcp chain
                            araw = wa.tile([P, SC], F32, tag="araw", bufs=2,
                                           name=f"araw{c}_{h}")
                            nc.vector.tensor_copy(araw[:], pvp[:])
                            rcp = wa.tile([1, SC], F32, tag="rcp", bufs=2)
                            nc.vector.reciprocal(rcp[:], rsp[:])
                            rcpb = wa.tile([P, SC], F32, tag="rcpb", bufs=2)
                            nc.gpsimd.partition_broadcast(rcpb[:], rcp[:])
                            att = wa.tile([P, SC], BF16, tag="at", bufs=16,
                                          name=f"at{c}_{h}")
                            atl[(c, h)] = att
                            nc.vector.tensor_tensor(att[:], araw[:], rcpb[:],
                                                    ALU.mult)
                            yield

                    def g_o(c):
                        for m in range(NT_HID):
                            ps = psA.tile([P, SC], F32, tag="sc", bufs=4,
                                          name=f"o{c}_{m}")
                            for a in range(HQ):
                                nc.tensor.matmul(ps[:], wo_sb[:, a, m, :],
                                                 atl[(c, a)][:],
                                                 start=(a == 0),
                                                 stop=(a == HQ - 1))
                            ev = wa.tile([P, SC], BF16, tag="oev", bufs=3)
                            nc.vector.tensor_copy(ev[:], ps[:])
                            nc.sync.dma_start(rs1_in[c][m * P:(m + 1) * P, :],
                                              ev[:])
                            yield
                        nc.gpsimd.collective_compute(
                            "ReduceScatter", ALU.add, replica_groups=RG,
                            ins=[rs1_in[c][:].opt()], outs=[rs1_out[c][:].opt()])
                        yield

                    def g_r2(c):
                        # residual2 = o_proj partial-sum + x (Pool/scalar/sync
                        # engines only — must not HOL-block PE/DVE streams)
                        cs = slice(c * SC, (c + 1) * SC)
                        hh = c // 2
                        hcs = slice((c % 2) * SC, (c % 2 + 1) * SC)
                        ssq_acc = wr.tile([1, SC], F32, tag="ssqrow", bufs=2,
                                          name=f"ssq2_{c}")
                        for i in range(NT_HSH):
                            o_i = wr.tile([P, SC], BF16, tag="r2o", bufs=2)
                            nc.gpsimd.dma_start(
                                o_i[:], rs1_out[c][i * P:(i + 1) * P, :])
                            r2t = wr.tile([P, SC], F32, tag="p1x", bufs=3)
                            nc.gpsimd.tensor_tensor(r2t[:], o_i[:],
                                                    xown[c][:, i, :], ALU.add)
                            nc.sync.dma_start(out_res2[i * P:(i + 1) * P, cs],
                                              r2t[:])
                            r2b = wr.tile([P, SC], BF16, tag="r2b", bufs=2)
                            nc.vector.tensor_copy(r2b[:], r2t[:])
                            nc.sync.dma_start(
                                ag2_in[hh][i * P:(i + 1) * P, hcs], r2b[:])
                            sq = wr.tile([P, SC], F32, tag="p1s", bufs=2)
                            nc.vector.tensor_tensor(sq[:], r2t[:], r2t[:],
                                                    ALU.mult)
                            red = wr.tile([P, SC], F32, tag="p1r", bufs=2)
                            nc.gpsimd.partition_all_reduce(
                                red[:], sq[:], channels=P,
                                reduce_op=ReduceOp.add)
                            if i == 0:
                                nc.vector.tensor_copy(ssq_acc[:], red[:1, :])
                            else:
                                nc.vector.tensor_tensor(ssq_acc[:], ssq_acc[:],
                                                        red[:1, :], ALU.add)
                            yield
                        sb = wr.tile([1, SC], BF16, tag="ssqb", bufs=2)
                        nc.vector.tensor_copy(sb[:], ssq_acc[:])
                        nc.sync.dma_start(
                            ag2_in[hh][HID_SH:HID_SH + 1, hcs], sb[:])
                        if c % 2 == 1:
                            nc.gpsimd.collective_compute(
                                "AllGather", ALU.bypass, replica_groups=RG,
                                ins=[ag2_in[hh][:].opt()],
                                outs=[ag2_out[hh][:].opt()])
                        yield

                    # era 5: first chunk pair, self-pipelined scores
                    weave(g_attn(0, True), g_attn(1, True))
                    # era 6: second chunk pair + o(0,1) as PE filler
                    weave(g_attn(2, False), g_attn(3, False),
                          (g_o(0), 2), (g_o(1), 2))
                    # era 6b: residual2(0,1) (non-PE; RS1(0/1) already fired),
                    # then o(2,3); r2(2,3) is deferred into the up(0) weave
                    weave(g_r2(0), g_r2(1))
                    weave(g_o(2), g_o(3))
                    r2_late = [g_r2(2), g_r2(3)]

            # close attention-era pools before the MLP era
            era_a.close()

            # ============== era M: MLP ==============
            if True:
                    with tc.tile_pool(name="workM", bufs=1) as wm, \
                         tc.tile_pool(name="psM", bufs=1, space="PSUM") as psM:

                        s2b = wm.tile([P, S], BF16, name="s2b")
                        uT = wm.tile([P, NT_INT, S], BF16, name="uT")

                        def emit_s2(h):
                            tok = slice(h * SH, (h + 1) * SH)
                            srb = wm.tile([8, SH], BF16, tag="srb2", bufs=1)
                            nc.gpsimd.dma_start(srb[:], ag2_v[h][:, HID_SH, :])
                            srf = wm.tile([8, SH], F32, tag="srf2", bufs=1)
                            nc.vector.tensor_copy(srf[:], srb[:])
                            ssum = wm.tile([8, SH], F32, tag="ssum2", bufs=1)
                            nc.gpsimd.partition_all_reduce(
                                ssum[:], srf[:], channels=8,
                                reduce_op=ReduceOp.add)
                            var = wm.tile([1, SH], F32, tag="var2", bufs=1)
                            nc.scalar.activation(var[:], ssum[:1, :], AF.Copy,
                                                 scale=1.0 / HID, bias=EPS)
                            nc.vector.reciprocal(var[:], var[:])  # = scale^2
                            varb = wm.tile([1, SH], BF16, tag="var2b", bufs=1)
                            nc.vector.tensor_copy(varb[:], var[:])
                            nc.gpsimd.partition_broadcast(s2b[:, tok], varb[:])

                        def g_up(h):
                            emit_s2(h)
                            for t2 in range(2):
                                tcs = slice(t2 * SC, (t2 + 1) * SC)
                                h2g = wm.tile([P, NT_HID, SC], BF16,
                                              tag="h2g", bufs=1,
                                              name=f"h2g{h}_{t2}")
                                for cb in range(N_CORES):
                                    nc.scalar.dma_start(
                                        h2g[:, cb * NT_HSH:(cb + 1) * NT_HSH,
                                            :],
                                        ag2_v[h][cb, :HID_SH, tcs].rearrange(
                                            "(t p) s -> p t s", p=P))
                                tkc = slice(h * SH + t2 * SC,
                                            h * SH + (t2 + 1) * SC)
                                for it in range(NT_INT):
                                    wt = wm.tile([P, NT_HID, P], BF16,
                                                 tag="wup", bufs=2,
                                                 name=f"wup{h}_{t2}_{it}")
                                    nc.scalar.dma_start(wt[:], wup[it])
                                    ps = psM.tile([P, SC], F32, tag="up",
                                                  bufs=4,
                                                  name=f"up{h}_{t2}_{it}")
                                    for k in range(NT_HID):
                                        nc.tensor.matmul(
                                            ps[:], wt[:, k, :], h2g[:, k, :],
                                            start=(k == 0),
                                            stop=(k == NT_HID - 1))
                                        if k % 8 == 7:
                                            yield
                                    rl = wm.tile([P, SC], F32, tag="rl",
                                                 bufs=2)
                                    nc.scalar.activation(rl[:], ps[:],
                                                         AF.Relu)
                                    nc.vector.tensor_tensor(uT[:, it, tkc],
                                                            rl[:], rl[:],
                                                            ALU.mult)
                                    yield

                        def g_down(th):
                            tok0 = th * SH
                            for q in range(4):
                                for m8 in range(8):
                                    m = q * 8 + m8
                                    wt = wm.tile([P, NT_INT, P], BF16,
                                                 tag="wdn", bufs=2,
                                                 name=f"wdn{th}_{m}")
                                    nc.scalar.dma_start(wt[:], wdn[m])
                                    for t2 in range(2):
                                        tok = slice(tok0 + t2 * SC,
                                                    tok0 + (t2 + 1) * SC)
                                        ps = psM.tile([P, SC], F32, tag="dn",
                                                      bufs=2,
                                                      name=f"dn{th}_{m}_{t2}")
                                        for it in range(NT_INT):
                                            nc.tensor.matmul(
                                                ps[:], wt[:, it, :],
                                                uT[:, it, tok],
                                                start=(it == 0),
                                                stop=(it == NT_INT - 1))
                                        ev = wm.tile([P, SC], BF16, tag="dnev",
                                                     bufs=3)
                                        nc.vector.tensor_tensor(
                                            ev[:], ps[:], s2b[:, tok],
                                            ALU.mult)
                                        nc.sync.dma_start(
                                            rs2_in[q * 2 + th][
                                                m8 * P:(m8 + 1) * P,
                                                t2 * SC:(t2 + 1) * SC], ev[:])
                                        yield
                                g = q * 2 + th
                                nc.gpsimd.collective_compute(
                                    "ReduceScatter", ALU.add,
                                    replica_groups=RG,
                                    ins=[rs2_in[g][:].opt()],
                                    outs=[rs2_out[g][:].opt()])
                                nc.gpsimd.dma_start(
                                    out_mlp[q * P:(q + 1) * P,
                                            tok0:tok0 + SH], rs2_out[g][:])
                                yield

                        # era 7: up(half0); residual2(2,3) woven in after a
                        # solo warm-up so their late RS1-gated ops never sit
                        # ahead of up evictions in any queue
                        gu0 = g_up(0)
                        for _ in range(40):
                            next(gu0)
                        weave(gu0, *r2_late)
                        # era 8: up(half1) + down(token half0)
                        weave(g_up(1), g_down(0))
                        # era 9: down(token half1)
                        weave(g_down(1))

    nc.compile()
    return nc


def shard_inputs(positions, hidden_states, residual, qkv_w, o_w, up_w, down_w,
                 ln1_w, ln2_w):
    import ml_dtypes
    BF = ml_dtypes.bfloat16
    hTf = np.ascontiguousarray(np.asarray(hidden_states, np.float32)
                               .reshape(S, HID).T)
    rTf = np.ascontiguousarray(np.asarray(residual, np.float32)
                               .reshape(S, HID).T)
    pos = np.ascontiguousarray(np.asarray(positions).reshape(1, S))
    ln1 = np.asarray(ln1_w, np.float32)
    ln2 = np.asarray(ln2_w, np.float32)
    qkv_w = np.asarray(qkv_w, np.float32) * ln1[:, None]
    up_w = np.asarray(up_w, np.float32) * ln2[:, None]
    o_w = np.asarray(o_w, np.float32)
    down_w = np.asarray(down_w, np.float32)
    q_size = N_HEADS * DHEAD
    kv = N_KV * DHEAD
    in_maps = []
    for c in range(N_CORES):
        wq_c = np.concatenate([
            qkv_w[:, c * HQ * DHEAD:(c + 1) * HQ * DHEAD],
            qkv_w[:, q_size + c * DHEAD:q_size + (c + 1) * DHEAD],
            qkv_w[:, q_size + kv + c * DHEAD:q_size + kv + (c + 1) * DHEAD],
        ], axis=1)                                   # [4096, 768]
        wq_p = wq_c.reshape(NT_HID, P, NJ, P).transpose(2, 1, 0, 3) \
            .reshape(NJ, P, NT_HID * P)
        wo_c = o_w[c * HQ * DHEAD:(c + 1) * HQ * DHEAD, :]   # [512, 4096]
        wo_p = wo_c.reshape(HQ, P, NT_HID, P).transpose(1, 0, 2, 3) \
            .reshape(P, HQ * NT_HID * P)
        wup_c = up_w[:, c * INT_SH:(c + 1) * INT_SH]         # [4096, 2048]
        wup_p = wup_c.reshape(NT_HID, P, NT_INT, P).transpose(2, 1, 0, 3) \
            .reshape(NT_INT, P, NT_HID * P)
        wdn_c = down_w[c * INT_SH:(c + 1) * INT_SH, :]       # [2048, 4096]
        wdn_p = wdn_c.reshape(NT_INT, P, NT_HID, P).transpose(2, 1, 0, 3) \
            .reshape(NT_HID, P, NT_INT * P)
        in_maps.append({
            "hT": np.ascontiguousarray(hTf[c * HID_SH:(c + 1) * HID_SH]),
            "rT": np.ascontiguousarray(rTf[c * HID_SH:(c + 1) * HID_SH]),
            "positions": pos,
            "wqkv": np.ascontiguousarray(wq_p.astype(BF)),
            "wo": np.ascontiguousarray(wo_p.astype(BF)),
            "wup": np.ascontiguousarray(wup_p.astype(BF)),
            "wdn": np.ascontiguousarray(wdn_p.astype(BF)),
        })
    return in_maps


_CACHE = {}


def kernel(**inputs):
    from concourse.bass_utils import run_bass_kernel_spmd
    if "nc" not in _CACHE:
        _CACHE["nc"] = build_graph()
    nc = _CACHE["nc"]
    in_maps = shard_inputs(**{k: np.asarray(v) for k, v in inputs.items()})
    res = run_bass_kernel_spmd(nc, in_maps, core_ids=list(range(N_CORES)),
                               trace=False)
    res2T = np.concatenate([res.results[c]["res2T"] for c in range(N_CORES)],
                           axis=0)
    mlpT = np.empty((HID, S), np.float32)
    for c in range(N_CORES):
        mt = res.results[c]["mlpT"]
        for q in range(4):
            mlpT[q * 1024 + c * P:q * 1024 + c * P + P] = \
                mt[q * P:(q + 1) * P]
    mlp_out = np.ascontiguousarray(mlpT.T).reshape(1, S, HID)
    residual2 = np.ascontiguousarray(res2T.T).reshape(1, S, HID)
    return mlp_out, residual2
